# revision 64
# baseline (speedup 1.0000x reference)
"""Allegro-style GNN message passing on 8 TRN2 NeuronCores.

Strategy (v2 — minimal host->device bytes):
- Host: shard edges by SENDER node range (1024 nodes/core) -> sender
  segment-sums are fully core-local (no cross-core collectives).
  Within a core, group edges by 128-node sender windows; pad each
  (core, window) group to a common K_WIN with dummy edges (d=2 -> u=0 ->
  zero contribution).
- Inputs per core are just 3 packed blobs (~1.4 MB total): u8 index
  planes (sender-local / receiver-lo / receiver-hi), an f32 blob
  (edge vectors + biases + wcol), and a bf16 blob (node table +
  weights). One-hot scatter/gather matrices and endpoint-attribute
  gathers are built ON DEVICE (iota + is_equal + PE transposes +
  one-hot matmuls) instead of being shipped from the host -- the axon
  PJRT tunnel moves ~40 MB/s, so the previous 17.7 MB/core of host-
  built one-hots dominated wall time.
- Layer algebra: Y[:,0] == 1, so layer-1 only needs a 16-wide
  segment-sum of w1; W_lsh[1] output is dead; V1 is only needed at
  component 0 => contraction with Ytil = Y * W_lsh[0][:,0].
- Receiver scatter: node id = hi*128+lo; per edge-tile matmul with lo
  one-hot lhsT and (hi one-hot * edge_out) rhs accumulates [128,64]
  partials in PSUM; host sums the 8 per-core partials (the unshard).
- 1/sqrt(AVG_NEIGH) and the 1/sqrt(2) residual scales are folded into
  weights on the host.
"""
import math
import sys

import numpy as np

sys.path.insert(0, "/opt/trn_rl_repo")

import ml_dtypes  # noqa: E402

try:
    import jax
    jax.config.update("jax_compilation_cache_dir", "/tmp/jax_pcache")
    jax.config.update("jax_persistent_cache_min_entry_size_bytes", -1)
    jax.config.update("jax_persistent_cache_min_compile_time_secs", 0.0)
except Exception:
    pass

BF16 = ml_dtypes.bfloat16
SIM_SILU = False   # CoreSim lacks Silu; emulate with Sigmoid*z when set

N, E, MUL, H, F = 8192, 131072, 16, 256, 16
NB = 8
INV = 1.0 / math.sqrt(16.0)
NC = 8
NPC = N // NC          # nodes per core
WIN = 128
NW = NPC // WIN        # windows per core
RWIN = N // WIN        # 64 receiver windows
SQ = math.sqrt(0.5)

# ---- bf16 weight-blob column layout [128, CB] (replicated) ----
OFF_WE0 = 0                      # we0 [40, 256] rows 0..39; wly1fb_0
#   shares these cols at rows 64..79
OFF_WE1 = OFF_WE0 + 256          # we1 2 x [128, 256]
OFF_WV0 = OFF_WE1 + 512          # wv0 2 x [128, 16]
OFF_WLW0 = OFF_WV0 + 32
OFF_WLW1 = OFF_WLW0 + 32
OFF_WLY1 = (OFF_WLW1 + 32, OFF_WLW1 + 32 + 512)
OFF_WLY2 = (OFF_WLY1[1] + 512, OFF_WLY1[1] + 512 + 512)
OFF_WOUT = OFF_WLY2[1] + 512     # wout 2 x [128, 1]
OFF_FB1 = OFF_WOUT + 2           # wly1fb_1 [16, 256] (rows 0..15)
CB = OFF_FB1 + 256
# Everything ships in ONE int8 array per core (each extra array costs
# extra axon tunnel latency): index planes, int8 node table, then RAW
# BYTES of the int16 vector planes / bf16 weight blob / f32 misc,
# DMA'd on device via AP.bitcast — zero reconstruction ops, and bf16
# weight bytes keep their favorable wire compressibility.
NAT8 = 1024 + 128
VSCALE = 2.0 ** -14              # int16 fixed-point scale for vectors


def _host_shard(vectors, senders, receivers):
    """Group edges by (core, sender-window); pad to common K_WIN."""
    core = senders // NPC
    win = (senders % NPC) // WIN
    key = core * NW + win
    order = np.argsort(key, kind="stable")
    counts = np.bincount(key, minlength=NC * NW)
    kwin = int(((counts.max() + 127) // 128) * 128)
    starts = np.zeros(NC * NW + 1, np.int64)
    np.cumsum(counts, out=starts[1:])

    EP = NW * kwin
    shards = []
    for c in range(NC):
        v16 = np.zeros((EP, 3), np.int16)
        v16[:, 0] = 24576              # dummy edge: d = 1.5 -> u = 0
        sl = np.zeros(EP, np.int8)     # sender local-in-window
        rlo = np.zeros(EP, np.int8)
        rhi = np.zeros(EP, np.int8)
        for w in range(NW):
            g = c * NW + w
            eid = order[starts[g]:starts[g + 1]]
            o = w * kwin
            n_e = len(eid)
            v16[o:o + n_e] = np.round(vectors[eid] / VSCALE).astype(np.int16)
            sl[o:o + n_e] = (senders[eid] - (c * NPC + w * WIN)).astype(np.int8)
            rlo[o:o + n_e] = (receivers[eid] % 128).astype(np.int8)
            rhi[o:o + n_e] = (receivers[eid] // 128).astype(np.int8)
        shards.append((v16, sl, rlo, rhi))
    return kwin, shards


def _plane(a, T_ALL):
    """[EP] or [EP, k] -> plane layout [128, T_ALL*(k)] with e = t*128+p."""
    if a.ndim == 1:
        return np.ascontiguousarray(a.reshape(T_ALL, 128).T)
    # [EP, k] -> [128, k*T_ALL] with component-major column groups
    k = a.shape[1]
    p = a.reshape(T_ALL, 128, k).transpose(2, 1, 0)     # [k, 128, T_ALL]
    return np.ascontiguousarray(p.reshape(k * 128, T_ALL)).reshape(k, 128, T_ALL)


def _prep_weights(i):
    """Fold INV and residual 1/sqrt(2) scales into weights (f32)."""
    w = {}
    w["we0"] = i["W_e0"]                                       # [40,256]
    w["we1"] = i["W_e1"]
    w["wv0"] = i["W_v0"]
    w["wlw0"] = i["W_lw"][0] * INV
    w["wlw1"] = i["W_lw"][1] * INV * SQ                        # x1 = sq*x1'
    wly1_1 = i["W_ly1"][1].copy()
    wly1_1[:H] *= SQ                                           # x rows scaled
    w["wly1_0"] = i["W_ly1"][0]
    w["wly1_1"] = wly1_1
    w["wly2_0"] = i["W_ly2"][0]
    w["wly2_1"] = i["W_ly2"][1]
    w["wout"] = i["W_out"] * INV * 0.5                         # x2 = .5*x2'
    return w


def _pack_blobw(i):
    """Replicated f32 weight blob [128, CB] (quantized later)."""
    w = _prep_weights(i)
    blob = np.zeros((128, CB), np.float32)
    # rhs row order is [snd attrs(16), rcv attrs(16), bessel(8)] so the
    # on-device copies land on legal partition offsets (0 and 32)
    blob[0:40, OFF_WE0:OFF_WE0 + 256] = np.vstack([w["we0"][8:40],
                                                   w["we0"][0:8]])
    blob[64:80, OFF_WE0:OFF_WE0 + 256] = w["wly1_0"][256:272]
    blob[0:16, OFF_FB1:OFF_FB1 + 256] = w["wly1_1"][256:272]
    for kc in range(2):
        s = slice(kc * 128, (kc + 1) * 128)
        blob[:, OFF_WE1 + kc * 256:OFF_WE1 + (kc + 1) * 256] = w["we1"][s]
        blob[:, OFF_WV0 + kc * 16:OFF_WV0 + (kc + 1) * 16] = w["wv0"][s]
        blob[:, OFF_WLW0 + kc * 16:OFF_WLW0 + (kc + 1) * 16] = w["wlw0"][s]
        blob[:, OFF_WLW1 + kc * 16:OFF_WLW1 + (kc + 1) * 16] = w["wlw1"][s]
        blob[:, OFF_WOUT + kc:OFF_WOUT + kc + 1] = w["wout"][s]
    for l in range(2):
        m = w[f"wly1_{l}"]
        for kc in range(2):
            s = slice(kc * 128, (kc + 1) * 128)
            blob[:, OFF_WLY1[l] + kc * 256:OFF_WLY1[l] + (kc + 1) * 256] = m[s]
            blob[:, OFF_WLY2[l] + kc * 256:OFF_WLY2[l] + (kc + 1) * 256] = \
                w[f"wly2_{l}"][s]
    return blob


def make_in_maps(inputs):
    kwin, shards = _host_shard(inputs["vectors"], inputs["senders"],
                               inputs["receivers"])
    EP = NW * kwin
    T_ALL = EP // 128
    bias_list = [inputs["b_e0"], inputs["b_e1"],
                 inputs["b_ly1"][0], inputs["b_ly1"][1],
                 inputs["b_ly2"][0], inputs["b_ly2"][1]]
    wcol = inputs["W_lsh"][0][:, 0]                            # [16]
    blobw = _pack_blobw(inputs)
    na = inputs["node_attrs"]                                  # [N, F]
    nat_scale = float(np.abs(na).max() / 127.0)
    naq = np.round(na / nat_scale).clip(-127, 127).astype(np.int8)
    # nat8[lo, f*64+hi] = naq[hi*128+lo, f]
    nat = naq.reshape(RWIN, 128, F).transpose(1, 2, 0).reshape(128, 1024)
    wbytes = blobw.astype(BF16).view(np.int8)                  # [128, 2*CB]
    misc = np.zeros((128, 28), np.float32)
    for i, b in enumerate(bias_list):
        misc[:, 2 * i] = b[0:128]
        misc[:, 2 * i + 1] = b[128:256]
    misc[:, 12:28] = np.tile(wcol.reshape(1, 16), (128, 1))
    mbytes = misc.view(np.int8)                                # [128, 112]
    OV = 3 * T_ALL + NAT8
    OW = OV + 6 * T_ALL
    OM = OW + 2 * CB
    TOT = OM + 112
    in_maps = []
    dbg = []
    for c in range(NC):
        v16, sl, rlo, rhi = shards[c]
        b8 = np.empty((128, TOT), np.int8)
        b8[:, 0:T_ALL] = _plane(sl, T_ALL)
        b8[:, T_ALL:2 * T_ALL] = _plane(rlo, T_ALL)
        b8[:, 2 * T_ALL:3 * T_ALL] = _plane(rhi, T_ALL)
        b8[:, 3 * T_ALL:3 * T_ALL + 1024] = nat
        # snat[lo, w*16+f] = naq[(c*8+w)*128+lo, f]
        sn = naq.reshape(RWIN, 128, F)[c * NW:(c + 1) * NW]    # [w, lo, f]
        b8[:, 3 * T_ALL + 1024:3 * T_ALL + NAT8] = \
            sn.transpose(1, 0, 2).reshape(128, 128)
        vp = _plane(v16, T_ALL)                                # [3,128,T] i16
        vrow = np.ascontiguousarray(
            vp.transpose(1, 0, 2).reshape(128, 3 * T_ALL))     # [128, 3T] i16
        b8[:, OV:OV + 6 * T_ALL] = vrow.view(np.int8)
        b8[:, OW:OW + 2 * CB] = wbytes
        b8[:, OM:OM + 112] = mbytes
        in_maps.append({"blob8": b8})
        dbg.append(dict(vec=v16.astype(np.float32) * VSCALE,
                        sl=sl, rlo=rlo, rhi=rhi))
    return kwin, nat_scale, in_maps, dbg


_CAP_SKIP = {"InstEventSemaphore", "InstBranch", "InstNop",
             "InstCollectiveCompute"}
_CAP_LIMITS = {}


def _split_waits(nc, mybir, mk_carrier, limit=1):
    """Walrus codegen allows only 1 embedded sem-wait on compute
    instructions.  For each instruction with more, strip the extras onto
    freshly created same-engine carrier instructions inserted directly
    before it (engines are in-order, so this preserves semantics)."""
    f = nc.m.functions[0]
    made = 0
    for bb in f.blocks:
        insts = list(bb.instructions)
        plan = []          # (index, [carrier insts])
        for i, inst in enumerate(insts):
            tname = type(inst).__name__
            si = inst.sync_info
            nwait = len(si.on_wait) if (si and si.on_wait) else 0
            lim = _CAP_LIMITS.get(tname, limit)
            if tname in _CAP_SKIP or nwait <= lim:
                continue
            waits = list(si.on_wait)
            extras, keep = waits[:-lim], waits[-lim:]
            carriers = []
            for wt in extras:
                ci = mk_carrier(inst.engine)
                if ci is None:
                    keep.insert(0, wt)
                    continue
                ci.sync_info = mybir.SyncInfo(on_wait=[wt], on_update=[])
                carriers.append(ci)
                made += 1
            inst.sync_info = mybir.SyncInfo(on_wait=keep,
                                            on_update=si.on_update)
            if carriers:
                plan.append((i, carriers))
        if plan:
            new = []
            pmap = dict(plan)
            for i, inst in enumerate(insts):
                if i in pmap:
                    new.extend(pmap[i])
                new.append(inst)
            bb.instructions = new
    return made


def build_graph(kwin, nat_scale):
    from concourse import bass, mybir
    from concourse.masks import make_identity
    from concourse.tile import TileContext

    EP = NW * kwin
    T_ALL = EP // 128
    T_W = kwin // 128
    NCH = (kwin + 511) // 512      # free chunks per window
    OV = 3 * T_ALL + NAT8
    OW = OV + 6 * T_ALL
    OM = OW + 2 * CB
    TOT = OM + 112

    f32 = mybir.dt.float32
    bf16 = mybir.dt.bfloat16
    i32 = mybir.dt.int32
    i8 = mybir.dt.int8
    i16 = mybir.dt.int16
    AX = mybir.AxisListType.X
    OP = mybir.AluOpType
    AF = mybir.ActivationFunctionType

    nc = bass.Bass()
    carrier_sem_cm = nc.semaphore("carrier_sem")
    carrier_sem = carrier_sem_cm.__enter__()
    dp = nc.declare_dram_parameter
    d_b8 = dp("blob8", [128, TOT], i8, isOutput=False)
    d_out = dp("out", [128, RWIN], f32, isOutput=True)

    with TileContext(nc) as tc:
        with (
            tc.tile_pool(name="glob", bufs=1) as gp,
            tc.tile_pool(name="wgt", bufs=1) as wp,
            tc.tile_pool(name="win", bufs=2) as wnp,
            tc.tile_pool(name="big", bufs=1) as bgp,
            tc.tile_pool(name="sml", bufs=3) as sp,
            tc.tile_pool(name="ps_mlp", bufs=2, space="PSUM") as pmlp,
            tc.tile_pool(name="ps_acc", bufs=1, space="PSUM") as pacc,
            tc.tile_pool(name="ps_gth", bufs=1, space="PSUM") as pgth,
            tc.tile_pool(name="ps_sml", bufs=1, space="PSUM") as psml,
            tc.tile_pool(name="ps_rcv", bufs=1, space="PSUM") as prcv,
        ):
            # ---------------- blob to SBUF ----------------
            i8t = wp.tile([128, OV], i8, tag="i8t")
            nc.sync.dma_start(out=i8t[:], in_=d_b8[:, 0:OV])
            v16t = wp.tile([128, 3 * T_ALL], i16, tag="v16t")
            nc.sync.dma_start(out=v16t[:],
                              in_=d_b8[:, OV:OV + 6 * T_ALL].bitcast(i16))
            wb = wp.tile([128, CB], bf16, tag="wb")
            nc.sync.dma_start(out=wb[:],
                              in_=d_b8[:, OW:OW + 2 * CB].bitcast(bf16))
            fbt = wp.tile([128, 28], f32, tag="fbt")
            nc.sync.dma_start(out=fbt[:],
                              in_=d_b8[:, OM:OM + 112].bitcast(f32))
            slf = wp.tile([128, T_ALL], f32, tag="slf")
            rlof = wp.tile([128, T_ALL], f32, tag="rlof")
            rhif = wp.tile([128, T_ALL], f32, tag="rhif")
            nc.vector.tensor_copy(out=slf[:], in_=i8t[:, 0:T_ALL])
            nc.vector.tensor_copy(out=rlof[:], in_=i8t[:, T_ALL:2 * T_ALL])
            nc.vector.tensor_copy(out=rhif[:], in_=i8t[:, 2 * T_ALL:3 * T_ALL])
            # dequantized node table (nat 1024 cols + snat 128 cols)
            natbf = wp.tile([128, NAT8], bf16, tag="natbf")
            nc.vector.tensor_scalar(
                out=natbf[:], in0=i8t[:, 3 * T_ALL:3 * T_ALL + NAT8],
                scalar1=float(nat_scale), scalar2=None, op0=OP.mult)

            ident = wp.tile([128, 128], f32, tag="ident")
            make_identity(nc, ident[:])
            identb = wp.tile([128, 128], bf16, tag="identb")
            make_identity(nc, identb[:])
            it32 = wp.tile([128, 128], i32, tag="it32")
            nc.gpsimd.iota(out=it32[:], pattern=[[1, 128]], base=0,
                           channel_multiplier=0)
            iof = wp.tile([128, 128], f32, tag="iof")
            nc.vector.tensor_copy(out=iof[:], in_=it32[:])
            ones_bf = wp.tile([1, 128], bf16, tag="ones")
            nc.gpsimd.memset(ones_bf[:], 1.0)

            # views into the blobs
            def bias(i, hc):
                return fbt[:, 2 * i + hc, None]
            wcol = fbt[:, 12:28]
            natv = natbf[:, 0:1024]

            # ---------------- edge-scalar stage (planes [128,T_ALL]) ----
            ta = gp.tile([128, T_ALL], f32)
            tb = gp.tile([128, T_ALL], f32)
            ts = nc.vector.tensor_scalar
            act = nc.scalar.activation
            # dequantize int16 fixed-point vectors
            vx = gp.tile([128, T_ALL], f32)
            vy = gp.tile([128, T_ALL], f32)
            vz = gp.tile([128, T_ALL], f32)
            for j, vv in enumerate((vx, vy, vz)):
                ts(out=vv[:], in0=v16t[:, j * T_ALL:(j + 1) * T_ALL],
                   scalar1=VSCALE, scalar2=None, op0=OP.mult)

            def silu_act(out, ps_in, bias_ap):
                if not SIM_SILU:
                    act(out=out, in_=ps_in, func=AF.Silu, bias=bias_ap)
                else:
                    pp = ps_in.shape[0]
                    sg = bgp.tile([128, 512], f32, tag="simsilu")
                    zz_ = bgp.tile([128, 512], f32, tag="simsilu2")
                    cw_ = ps_in.shape[-1]
                    act(out=sg[:pp, :cw_], in_=ps_in, func=AF.Sigmoid,
                        bias=bias_ap)
                    nc.vector.tensor_scalar(out=zz_[:pp, :cw_], in0=ps_in,
                                            scalar1=bias_ap, scalar2=None,
                                            op0=OP.add)
                    nc.vector.tensor_mul(out=out, in0=sg[:pp, :cw_],
                                         in1=zz_[:pp, :cw_])
            d_pl = gp.tile([128, T_ALL], f32)
            nc.vector.tensor_mul(out=ta[:], in0=vx[:], in1=vx[:])
            nc.vector.tensor_mul(out=tb[:], in0=vy[:], in1=vy[:])
            nc.vector.tensor_add(out=ta[:], in0=ta[:], in1=tb[:])
            nc.vector.tensor_mul(out=tb[:], in0=vz[:], in1=vz[:])
            nc.vector.tensor_add(out=ta[:], in0=ta[:], in1=tb[:])
            act(out=d_pl[:], in_=ta[:], func=AF.Sqrt)
            rinv = gp.tile([128, T_ALL], f32)
            nc.vector.reciprocal(out=rinv[:], in_=d_pl[:])
            ux = gp.tile([128, T_ALL], f32)
            uy = gp.tile([128, T_ALL], f32)
            uz = gp.tile([128, T_ALL], f32)
            nc.vector.tensor_mul(out=ux[:], in0=vx[:], in1=rinv[:])
            nc.vector.tensor_mul(out=uy[:], in0=vy[:], in1=rinv[:])
            nc.vector.tensor_mul(out=uz[:], in0=vz[:], in1=rinv[:])

            # envelope u = 1 + d^6*(-28 + 48d - 21d^2), zero for d >= 1
            u_pl = gp.tile([128, T_ALL], f32)
            nc.vector.tensor_mul(out=ta[:], in0=d_pl[:], in1=d_pl[:])   # d2
            nc.vector.tensor_mul(out=tb[:], in0=ta[:], in1=d_pl[:])     # d3
            nc.vector.tensor_mul(out=tb[:], in0=tb[:], in1=tb[:])       # d6
            ts(out=ta[:], in0=ta[:], scalar1=-21.0, scalar2=None, op0=OP.mult)
            tc_q = gp.tile([128, T_ALL], f32)
            ts(out=tc_q[:], in0=d_pl[:], scalar1=48.0, scalar2=-28.0,
               op0=OP.mult, op1=OP.add)
            nc.vector.tensor_add(out=ta[:], in0=ta[:], in1=tc_q[:])
            nc.vector.tensor_mul(out=tb[:], in0=tb[:], in1=ta[:])
            ts(out=tb[:], in0=tb[:], scalar1=1.0, scalar2=None, op0=OP.add)
            ts(out=ta[:], in0=d_pl[:], scalar1=1.0, scalar2=None,
               op0=OP.is_lt)
            nc.vector.tensor_mul(out=u_pl[:], in0=tb[:], in1=ta[:])

            # spherical harmonics Y [128, T_ALL, 16] f32
            Yt = gp.tile([128, T_ALL, 16], f32)
            s3 = 3.0 ** 0.5; s5 = 5.0 ** 0.5; s15 = 15.0 ** 0.5
            s7 = 7.0 ** 0.5
            c33 = (35.0 / 8.0) ** 0.5; c32 = 105.0 ** 0.5
            c31 = (21.0 / 8.0) ** 0.5
            xx = gp.tile([128, T_ALL], f32)
            yy = gp.tile([128, T_ALL], f32)
            zz = gp.tile([128, T_ALL], f32)
            xy = gp.tile([128, T_ALL], f32)
            nc.vector.tensor_mul(out=xx[:], in0=ux[:], in1=ux[:])
            nc.vector.tensor_mul(out=yy[:], in0=uy[:], in1=uy[:])
            nc.vector.tensor_mul(out=zz[:], in0=uz[:], in1=uz[:])
            nc.vector.tensor_mul(out=xy[:], in0=ux[:], in1=uy[:])
            ts(out=Yt[:, :, 0], in0=ux[:], scalar1=0.0, scalar2=1.0,
               op0=OP.mult, op1=OP.add)
            ts(out=Yt[:, :, 1], in0=ux[:], scalar1=s3, scalar2=None,
               op0=OP.mult)
            ts(out=Yt[:, :, 2], in0=uy[:], scalar1=s3, scalar2=None,
               op0=OP.mult)
            ts(out=Yt[:, :, 3], in0=uz[:], scalar1=s3, scalar2=None,
               op0=OP.mult)
            ts(out=Yt[:, :, 4], in0=xy[:], scalar1=s15, scalar2=None,
               op0=OP.mult)
            nc.vector.tensor_mul(out=ta[:], in0=uy[:], in1=uz[:])
            ts(out=Yt[:, :, 5], in0=ta[:], scalar1=s15, scalar2=None,
               op0=OP.mult)
            ts(out=Yt[:, :, 6], in0=zz[:], scalar1=1.5 * s5,
               scalar2=-0.5 * s5, op0=OP.mult, op1=OP.add)
            nc.vector.tensor_mul(out=tb[:], in0=ux[:], in1=uz[:])
            ts(out=Yt[:, :, 7], in0=tb[:], scalar1=s15, scalar2=None,
               op0=OP.mult)
            xmy = gp.tile([128, T_ALL], f32)
            nc.vector.tensor_sub(out=xmy[:], in0=xx[:], in1=yy[:])
            ts(out=Yt[:, :, 8], in0=xmy[:], scalar1=0.5 * s15, scalar2=None,
               op0=OP.mult)
            ts(out=ta[:], in0=xx[:], scalar1=3.0, scalar2=None, op0=OP.mult)
            nc.vector.tensor_sub(out=ta[:], in0=ta[:], in1=yy[:])
            nc.vector.tensor_mul(out=ta[:], in0=ta[:], in1=uy[:])
            ts(out=Yt[:, :, 9], in0=ta[:], scalar1=c33, scalar2=None,
               op0=OP.mult)
            nc.vector.tensor_mul(out=ta[:], in0=xy[:], in1=uz[:])
            ts(out=Yt[:, :, 10], in0=ta[:], scalar1=c32, scalar2=None,
               op0=OP.mult)
            ts(out=ta[:], in0=zz[:], scalar1=5.0, scalar2=-1.0,
               op0=OP.mult, op1=OP.add)
            nc.vector.tensor_mul(out=tb[:], in0=ta[:], in1=uy[:])
            ts(out=Yt[:, :, 11], in0=tb[:], scalar1=c31, scalar2=None,
               op0=OP.mult)
            nc.vector.tensor_mul(out=tb[:], in0=ta[:], in1=ux[:])
            ts(out=Yt[:, :, 13], in0=tb[:], scalar1=c31, scalar2=None,
               op0=OP.mult)
            nc.vector.tensor_mul(out=ta[:], in0=zz[:], in1=uz[:])
            ts(out=ta[:], in0=ta[:], scalar1=2.5 * s7, scalar2=None,
               op0=OP.mult)
            ts(out=tb[:], in0=uz[:], scalar1=1.5 * s7, scalar2=None,
               op0=OP.mult)
            nc.vector.tensor_sub(out=Yt[:, :, 12], in0=ta[:], in1=tb[:])
            nc.vector.tensor_mul(out=ta[:], in0=xmy[:], in1=uz[:])
            ts(out=Yt[:, :, 14], in0=ta[:], scalar1=0.5 * c32, scalar2=None,
               op0=OP.mult)
            ts(out=ta[:], in0=yy[:], scalar1=3.0, scalar2=None, op0=OP.mult)
            nc.vector.tensor_sub(out=ta[:], in0=xx[:], in1=ta[:])
            nc.vector.tensor_mul(out=ta[:], in0=ta[:], in1=ux[:])
            ts(out=Yt[:, :, 15], in0=ta[:], scalar1=c33, scalar2=None,
               op0=OP.mult)

            # bessel (range-reduced): besu [128, T_ALL, 8]
            besu = gp.tile([128, T_ALL, 8], f32)
            rs = gp.tile([128, T_ALL], f32)
            ts(out=rs[:], in0=rinv[:], scalar1=math.sqrt(2.0), scalar2=None,
               op0=OP.mult)
            mi = gp.tile([128, T_ALL], mybir.dt.int32)
            for k in range(1, NB + 1):
                ts(out=ta[:], in0=d_pl[:], scalar1=0.5 * k, scalar2=None,
                   op0=OP.mult)
                nc.vector.tensor_copy(out=mi[:], in_=ta[:])
                nc.vector.tensor_copy(out=tb[:], in_=mi[:])
                nc.vector.tensor_sub(out=ta[:], in0=ta[:], in1=tb[:])
                # ta = frac in (-0.5, 1) whether the cast rounds or truncates
                ts(out=tb[:], in0=ta[:], scalar1=0.5, scalar2=None,
                   op0=OP.is_gt)
                nc.vector.tensor_sub(out=ta[:], in0=ta[:], in1=tb[:])
                act(out=ta[:], in_=ta[:], func=AF.Sin, scale=2.0 * math.pi)
                nc.vector.tensor_mul(out=besu[:, :, k - 1], in0=ta[:],
                                      in1=rs[:])

            # ---------------- persistent receiver accumulator ----------
            ps_rcv = prcv.tile([128, RWIN], f32, space="PSUM")

            # ---------------- window loop ----------------
            for w in range(NW):
                t0 = w * T_W
                ohs = wnp.tile([128, T_W, 128], bf16)   # [e, n]
                ohg = wnp.tile([128, T_W, 128], bf16)   # [n, e]
                rqs = wnp.tile([128, T_W, 128], bf16)   # [e, lo]
                rqg = wnp.tile([128, T_W, 128], bf16)   # [lo, e]
                rwt = wnp.tile([128, T_W, RWIN], bf16)  # [e, hi]
                xfm = wnp.tile([40, kwin], bf16)        # snd(16)+rcv(16)+bes(8)
                ufm = wnp.tile([1, kwin], bf16)
                tt = nc.vector.tensor_tensor
                tsw = slice(t0, t0 + T_W)
                tt(out=ohs[:],
                   in0=slf[:, tsw, None].to_broadcast([128, T_W, 128]),
                   in1=iof[:, None, :].to_broadcast([128, T_W, 128]),
                   op=OP.is_equal)
                tt(out=rqs[:],
                   in0=rlof[:, tsw, None].to_broadcast([128, T_W, 128]),
                   in1=iof[:, None, :].to_broadcast([128, T_W, 128]),
                   op=OP.is_equal)
                tt(out=rwt[:],
                   in0=rhif[:, tsw, None].to_broadcast([128, T_W, RWIN]),
                   in1=iof[:, None, 0:RWIN].to_broadcast([128, T_W, RWIN]),
                   op=OP.is_equal)
                for t in range(T_W):
                    tg = t0 + t
                    csl = slice(t * 128, (t + 1) * 128)
                    ptr = psml.tile([128, 128], bf16, space="PSUM", tag="trn")
                    nc.tensor.transpose(out=ptr[:], in_=ohs[:, t, :],
                                        identity=identb[:])
                    nc.vector.tensor_copy(out=ohg[:, t, :], in_=ptr[:])
                    ptr2 = psml.tile([128, 128], bf16, space="PSUM", tag="trn")
                    nc.tensor.transpose(out=ptr2[:], in_=rqs[:, t, :],
                                        identity=identb[:])
                    nc.vector.tensor_copy(out=rqg[:, t, :], in_=ptr2[:])
                    # bessel + u feature-major
                    pst = psml.tile([32, 128], f32, space="PSUM", tag="sml")
                    nc.tensor.transpose(out=pst[0:8, :], in_=besu[:, tg, :],
                                        identity=ident[:])
                    nc.vector.tensor_copy(out=xfm[32:40, csl],
                                          in_=pst[0:8, :])
                    psu1 = psml.tile([32, 128], f32, space="PSUM", tag="sml")
                    nc.tensor.transpose(out=psu1[0:1, :],
                                        in_=u_pl[:, tg, None],
                                        identity=ident[:])
                    nc.vector.tensor_copy(out=ufm[:, csl], in_=psu1[0:1, :])
                    # endpoint-attr gather: sender (window-local one-hot)
                    gcmb = sp.tile([128, 32], f32, tag="gcmb")
                    psn = psml.tile([128, 32], f32, space="PSUM", tag="sm2")
                    nc.tensor.matmul(
                        out=psn[:, 0:16], lhsT=ohg[:, t, :],
                        rhs=natbf[:, 1024 + w * 16:1024 + (w + 1) * 16],
                        start=True, stop=True)
                    nc.vector.tensor_copy(out=gcmb[:, 0:16], in_=psn[:, 0:16])
                    # receiver: lo-gather matmul then hi-select
                    for c2 in range(2):
                        prg = pgth.tile([128, 512], f32, space="PSUM",
                                        tag="gth")
                        nc.tensor.matmul(
                            out=prg[:], lhsT=rqg[:, t, :],
                            rhs=natv[:, c2 * 512:(c2 + 1) * 512],
                            start=True, stop=True)
                        prod = sp.tile([128, 8, RWIN], f32, tag="rsel")
                        nc.vector.tensor_mul(
                            out=prod[:],
                            in0=prg[:].rearrange("p (a b) -> p a b", b=RWIN),
                            in1=rwt[:, t, None, :].to_broadcast(
                                [128, 8, RWIN]))
                        nc.vector.reduce_sum(
                            out=gcmb[:, 16 + c2 * 8:16 + (c2 + 1) * 8, None],
                            in_=prod[:], axis=AX)
                    ptg = psml.tile([32, 128], f32, space="PSUM", tag="sml")
                    nc.tensor.transpose(out=ptg[:], in_=gcmb[:],
                                        identity=ident[:])
                    nc.vector.tensor_copy(out=xfm[0:32, csl], in_=ptg[:])

                # broadcast u row -> [128, kwin] bf16
                ubc = bgp.tile([128, kwin], bf16)
                for ch in range(NCH):
                    c0 = ch * 512
                    c1 = min(kwin, c0 + 512)
                    psu = pmlp.tile([128, 512], f32, space="PSUM", tag="mlp")
                    nc.tensor.matmul(out=psu[:, :c1 - c0], lhsT=ones_bf[:],
                                     rhs=ufm[:, c0:c1],
                                     start=True, stop=True)
                    nc.vector.tensor_copy(out=ubc[:, c0:c1],
                                          in_=psu[:, :c1 - c0])

                # ---- edge MLP: x0 = u*silu(e1(silu(e0(bes,attrs)))) ----
                x0 = bgp.tile([128, 2, kwin], bf16)
                th = bgp.tile([128, 2, kwin], bf16)
                for ch in range(NCH):
                    c0 = ch * 512
                    c1 = min(kwin, c0 + 512)
                    cw = c1 - c0
                    for hc in range(2):
                        ps = pmlp.tile([128, 512], f32, space="PSUM", tag="mlp")
                        nc.tensor.matmul(
                            out=ps[:, :cw],
                            lhsT=wb[0:40, OFF_WE0 + hc * 128:
                                    OFF_WE0 + (hc + 1) * 128],
                            rhs=xfm[:, c0:c1], start=True, stop=True)
                        silu_act(th[:, hc, c0:c1], ps[:, :cw], bias(0, hc))
                for ch in range(NCH):
                    c0 = ch * 512
                    c1 = min(kwin, c0 + 512)
                    cw = c1 - c0
                    for hc in range(2):
                        ps = pmlp.tile([128, 512], f32, space="PSUM", tag="mlp")
                        for kc in range(2):
                            nc.tensor.matmul(
                                out=ps[:, :cw],
                                lhsT=wb[:, OFF_WE1 + kc * 256 + hc * 128:
                                        OFF_WE1 + kc * 256 + (hc + 1) * 128],
                                rhs=th[:, kc, c0:c1],
                                start=(kc == 0), stop=(kc == 1))
                        silu_act(x0[:, hc, c0:c1], ps[:, :cw], bias(1, hc))
                for hc in range(2):
                    nc.vector.tensor_mul(out=x0[:, hc, :], in0=x0[:, hc, :],
                                          in1=ubc[:])

                # ---- xv, w0 (edge-major [128,16] per tile) ----
                xv = wnp.tile([128, T_W, MUL], f32)
                w0 = wnp.tile([128, T_W, MUL], bf16)
                for t in range(T_W):
                    tsl = slice(t * 128, (t + 1) * 128)
                    p12 = psml.tile([128, 32], f32, space="PSUM", tag="sm2")
                    for kc in range(2):
                        nc.tensor.matmul(
                            out=p12[:, 0:16], lhsT=x0[:, kc, tsl],
                            rhs=wb[:, OFF_WV0 + kc * 16:OFF_WV0 + (kc + 1) * 16],
                            start=(kc == 0), stop=(kc == 1))
                    for kc in range(2):
                        nc.tensor.matmul(
                            out=p12[:, 16:32], lhsT=x0[:, kc, tsl],
                            rhs=wb[:, OFF_WLW0 + kc * 16:
                                    OFF_WLW0 + (kc + 1) * 16],
                            start=(kc == 0), stop=(kc == 1))
                    nc.vector.tensor_copy(out=xv[:, t, :], in_=p12[:, 0:16])
                    nc.vector.tensor_copy(out=w0[:, t, :], in_=p12[:, 16:32])

                # ---- layer-0 scatter: wY[n, m*16+i] ----
                ps_acc = pacc.tile([128, 256], f32, space="PSUM", tag="acc")
                for t in range(T_W):
                    v2 = sp.tile([128, MUL, 16], bf16, tag="v2")
                    nc.vector.tensor_mul(
                        out=v2[:],
                        in0=w0[:, t, :, None].to_broadcast([128, MUL, 16]),
                        in1=Yt[:, t0 + t, None, :].to_broadcast(
                            [128, MUL, 16]))
                    nc.tensor.matmul(
                        out=ps_acc[:],
                        lhsT=ohs[:, t, :],
                        rhs=v2[:].rearrange("p a b -> p (a b)"),
                        start=(t == 0), stop=(t == T_W - 1))
                wY = wnp.tile([128, 256], bf16)
                nc.vector.tensor_copy(out=wY[:], in_=ps_acc[:])

                # ---- gather + Ytil contraction + feedback ----
                # fbfm lives at partitions 64..79 so its matmul shares the
                # base partition of the packed wly1fb_0 weights
                V10 = wnp.tile([128, T_W, MUL], f32)
                fbfm = wnp.tile([80, kwin], bf16)
                prod = wnp.tile([128, MUL, 16], f32)
                ytil = wnp.tile([128, MUL], f32)
                Ssb = wnp.tile([128, MUL], f32)
                fb = wnp.tile([128, MUL], f32)
                for t in range(T_W):
                    pgf = pgth.tile([128, 512], f32, space="PSUM", tag="gth")
                    pg = pgf[:, 0:256]
                    nc.tensor.matmul(out=pg, lhsT=ohg[:, t, :], rhs=wY[:],
                                     start=True, stop=True)
                    pg3 = pg.rearrange("p (a b) -> p a b", b=16)
                    nc.vector.tensor_mul(out=ytil[:], in0=Yt[:, t0 + t, :],
                                          in1=wcol)
                    nc.vector.tensor_mul(
                        out=prod[:], in0=pg3,
                        in1=ytil[:, None, :].to_broadcast([128, MUL, 16]))
                    nc.vector.reduce_sum(out=Ssb[:, :, None], in_=prod[:],
                                         axis=AX)
                    nc.vector.tensor_mul(out=V10[:, t, :], in0=Ssb[:],
                                          in1=xv[:, t, :])
                    nc.vector.tensor_mul(out=fb[:], in0=pg3[:, :, 0],
                                          in1=xv[:, t, :])
                    pst = psml.tile([32, 128], f32, space="PSUM", tag="sml")
                    nc.tensor.transpose(out=pst[0:16, :], in_=fb[:],
                                        identity=ident[:])
                    nc.vector.tensor_copy(
                        out=fbfm[64:80, t * 128:(t + 1) * 128],
                        in_=pst[0:16, :])

                # ---- layer-0 ly1/ly2 + residual -> x1 ----
                x1 = bgp.tile([128, 2, kwin], bf16)

                def mlp_block(xin, xout, l, fbrow, resid_sq2):
                    b1 = OFF_WLY1[l]
                    # wly1fb: layer 0 at rows 64..79 of the WE0 cols,
                    # layer 1 at rows 0..15 of its own FB1 cols
                    fbp, fbc = (64, OFF_WE0) if l == 0 else (0, OFF_FB1)
                    b2 = OFF_WLY2[l]
                    ty = bgp.tile([128, 2, kwin], bf16)
                    for ch in range(NCH):
                        c0 = ch * 512
                        c1 = min(kwin, c0 + 512)
                        cw = c1 - c0
                        for hc in range(2):
                            hs = slice(hc * 128, (hc + 1) * 128)
                            ps = pmlp.tile([128, 512], f32, space="PSUM",
                                           tag="mlp")
                            for kc in range(2):
                                nc.tensor.matmul(
                                    out=ps[:, :cw],
                                    lhsT=wb[:, b1 + kc * 256 + hc * 128:
                                            b1 + kc * 256 + (hc + 1) * 128],
                                    rhs=xin[:, kc, c0:c1],
                                    start=(kc == 0), stop=False)
                            nc.tensor.matmul(
                                out=ps[:, :cw],
                                lhsT=wb[fbp:fbp + 16, fbc + hc * 128:
                                        fbc + (hc + 1) * 128],
                                rhs=fbrow[fbp:fbp + 16, c0:c1],
                                start=False, stop=True)
                            silu_act(ty[:, hc, c0:c1], ps[:, :cw],
                                     bias(2 + l, hc))
                    ty2 = bgp.tile([128, 2, kwin], bf16)
                    for ch in range(NCH):
                        c0 = ch * 512
                        c1 = min(kwin, c0 + 512)
                        cw = c1 - c0
                        for hc in range(2):
                            ps = pmlp.tile([128, 512], f32, space="PSUM",
                                           tag="mlp")
                            for kc in range(2):
                                nc.tensor.matmul(
                                    out=ps[:, :cw],
                                    lhsT=wb[:, b2 + kc * 256 + hc * 128:
                                            b2 + kc * 256 + (hc + 1) * 128],
                                    rhs=ty[:, kc, c0:c1],
                                    start=(kc == 0), stop=(kc == 1))
                            silu_act(ty2[:, hc, c0:c1], ps[:, :cw],
                                     bias(4 + l, hc))
                    # x_out' = x_in' + s * u * y   (s = 1 or sqrt(2))
                    for hc in range(2):
                        nc.vector.tensor_mul(out=ty2[:, hc, :],
                                              in0=ty2[:, hc, :], in1=ubc[:])
                        if resid_sq2:
                            ts(out=ty2[:, hc, :], in0=ty2[:, hc, :],
                               scalar1=math.sqrt(2.0), scalar2=None,
                               op0=OP.mult)
                        nc.vector.tensor_add(out=xout[:, hc, :],
                                             in0=xin[:, hc, :],
                                             in1=ty2[:, hc, :])

                mlp_block(x0, x1, 0, fbfm, False)

                # ---- layer 1: w1, 16-wide scatter/gather, feedback ----
                w1 = wnp.tile([128, T_W, MUL], bf16)
                for t in range(T_W):
                    tsl = slice(t * 128, (t + 1) * 128)
                    p1 = psml.tile([128, 32], f32, space="PSUM", tag="sm2")
                    for kc in range(2):
                        nc.tensor.matmul(
                            out=p1[:, 0:MUL], lhsT=x1[:, kc, tsl],
                            rhs=wb[:, OFF_WLW1 + kc * 16:
                                    OFF_WLW1 + (kc + 1) * 16],
                            start=(kc == 0), stop=(kc == 1))
                    nc.vector.tensor_copy(out=w1[:, t, :], in_=p1[:, 0:MUL])
                ps_a1 = pacc.tile([128, 256], f32, space="PSUM", tag="acc")
                for t in range(T_W):
                    nc.tensor.matmul(out=ps_a1[:, 0:MUL], lhsT=ohs[:, t, :],
                                     rhs=w1[:, t, :],
                                     start=(t == 0), stop=(t == T_W - 1))
                wY1 = wnp.tile([128, MUL], bf16)
                nc.vector.tensor_copy(out=wY1[:], in_=ps_a1[:, 0:MUL])
                fbfm1 = wnp.tile([MUL, kwin], bf16)
                fb1 = wnp.tile([128, MUL], f32)
                for t in range(T_W):
                    pg = pgth.tile([128, 512], f32, space="PSUM", tag="gth")
                    nc.tensor.matmul(out=pg[:, 0:MUL], lhsT=ohg[:, t, :],
                                     rhs=wY1[:], start=True, stop=True)
                    nc.vector.tensor_mul(out=fb1[:], in0=pg[:, 0:MUL],
                                          in1=V10[:, t, :])
                    pst = psml.tile([32, 128], f32, space="PSUM", tag="sml")
                    nc.tensor.transpose(out=pst[0:16, :], in_=fb1[:],
                                        identity=ident[:])
                    nc.vector.tensor_copy(
                        out=fbfm1[:, t * 128:(t + 1) * 128],
                        in_=pst[0:16, :])

                # ---- layer-1 ly1/ly2 + residual -> x2 ----
                x2 = bgp.tile([128, 2, kwin], bf16)
                mlp_block(x1, x2, 1, fbfm1, True)

                # ---- edge out + receiver scatter ----
                eo = wnp.tile([128, 1], f32)
                mt = wnp.tile([128, RWIN], bf16)
                for t in range(T_W):
                    tsl = slice(t * 128, (t + 1) * 128)
                    p1 = psml.tile([128, 32], f32, space="PSUM", tag="sm2")
                    for kc in range(2):
                        nc.tensor.matmul(
                            out=p1[:, 0:1], lhsT=x2[:, kc, tsl],
                            rhs=wb[:, OFF_WOUT + kc:OFF_WOUT + kc + 1],
                            start=(kc == 0), stop=(kc == 1))
                    nc.vector.tensor_mul(out=eo[:], in0=p1[:, 0:1],
                                          in1=u_pl[:, t0 + t, None])
                    nc.vector.tensor_mul(
                        out=mt[:], in0=rwt[:, t, :],
                        in1=eo[:].to_broadcast([128, RWIN]))
                    nc.tensor.matmul(out=ps_rcv[:], lhsT=rqs[:, t, :],
                                     rhs=mt[:],
                                     start=(w == 0 and t == 0),
                                     stop=(w == NW - 1 and t == T_W - 1))

            out_sb = gp.tile([128, RWIN], f32)
            nc.vector.tensor_copy(out=out_sb[:], in_=ps_rcv[:])
            nc.sync.dma_start(out=d_out[:], in_=out_sb[:])

    ET = mybir.EngineType
    eng_map = {ET.DVE: nc.vector, ET.Activation: nc.scalar,
               ET.Pool: nc.gpsimd, ET.PE: nc.tensor, ET.SP: nc.sync}

    def mk_carrier(eng):
        be = eng_map.get(eng)
        if be is None:
            return None
        w = be.wait_ge(carrier_sem, 0)
        ci = w.ins if hasattr(w, "ins") else w
        for bb in nc.m.functions[0].blocks:
            il = list(bb.instructions)
            if any(x is ci for x in il):
                bb.instructions = [x for x in il if x is not ci]
                break
        return ci

    made = _split_waits(nc, mybir, mk_carrier)
    print(f"split_waits: carriers={made}", flush=True)
    return nc


def kernel(**inputs):
    inputs = {k: np.asarray(v) for k, v in inputs.items()}
    kwin, nat_scale, in_maps, _ = make_in_maps(inputs)
    nc = build_graph(kwin, nat_scale)
    from concourse.bass_utils import run_bass_kernel_spmd
    res = run_bass_kernel_spmd(nc, in_maps, core_ids=list(range(NC)))
    out = np.zeros((128, RWIN), np.float64)
    for r in res.results:
        out += np.asarray(r["out"], np.float64)
    # node n = hi*128 + lo stored at [lo, hi]
    return np.ascontiguousarray(out.T.reshape(N, 1)).astype(np.float32)


# revision 65
# speedup vs baseline: 1.0940x; 1.0940x over previous
"""Allegro-style GNN message passing on 8 TRN2 NeuronCores.

Strategy (v2 — minimal host->device bytes):
- Host: shard edges by SENDER node range (1024 nodes/core) -> sender
  segment-sums are fully core-local (no cross-core collectives).
  Within a core, group edges by 128-node sender windows; pad each
  (core, window) group to a common K_WIN with dummy edges (d=2 -> u=0 ->
  zero contribution).
- Inputs per core are just 3 packed blobs (~1.4 MB total): u8 index
  planes (sender-local / receiver-lo / receiver-hi), an f32 blob
  (edge vectors + biases + wcol), and a bf16 blob (node table +
  weights). One-hot scatter/gather matrices and endpoint-attribute
  gathers are built ON DEVICE (iota + is_equal + PE transposes +
  one-hot matmuls) instead of being shipped from the host -- the axon
  PJRT tunnel moves ~40 MB/s, so the previous 17.7 MB/core of host-
  built one-hots dominated wall time.
- Layer algebra: Y[:,0] == 1, so layer-1 only needs a 16-wide
  segment-sum of w1; W_lsh[1] output is dead; V1 is only needed at
  component 0 => contraction with Ytil = Y * W_lsh[0][:,0].
- Receiver scatter: node id = hi*128+lo; per edge-tile matmul with lo
  one-hot lhsT and (hi one-hot * edge_out) rhs accumulates [128,64]
  partials in PSUM; host sums the 8 per-core partials (the unshard).
- 1/sqrt(AVG_NEIGH) and the 1/sqrt(2) residual scales are folded into
  weights on the host.
"""
import math
import sys

import numpy as np

sys.path.insert(0, "/opt/trn_rl_repo")

import ml_dtypes  # noqa: E402

try:
    import jax
    jax.config.update("jax_compilation_cache_dir", "/tmp/jax_pcache")
    jax.config.update("jax_persistent_cache_min_entry_size_bytes", -1)
    jax.config.update("jax_persistent_cache_min_compile_time_secs", 0.0)
except Exception:
    pass

BF16 = ml_dtypes.bfloat16
SIM_SILU = False   # CoreSim lacks Silu; emulate with Sigmoid*z when set

N, E, MUL, H, F = 8192, 131072, 16, 256, 16
NB = 8
INV = 1.0 / math.sqrt(16.0)
NC = 8
NPC = N // NC          # nodes per core
WIN = 128
NW = NPC // WIN        # windows per core
RWIN = N // WIN        # 64 receiver windows
SQ = math.sqrt(0.5)

# ---- bf16 weight-blob column layout [128, CB] (replicated) ----
OFF_WE0 = 0                      # we0 [40, 256] rows 0..39; wly1fb_0
#   shares these cols at rows 64..79
OFF_WE1 = OFF_WE0 + 256          # we1 2 x [128, 256]
OFF_WV0 = OFF_WE1 + 512          # wv0 2 x [128, 16]
OFF_WLW0 = OFF_WV0 + 32
OFF_WLW1 = OFF_WLW0 + 32
OFF_WLY1 = (OFF_WLW1 + 32, OFF_WLW1 + 32 + 512)
OFF_WLY2 = (OFF_WLY1[1] + 512, OFF_WLY1[1] + 512 + 512)
OFF_WOUT = OFF_WLY2[1] + 512     # wout 2 x [128, 1]
OFF_FB1 = OFF_WOUT + 2           # wly1fb_1 [16, 256] (rows 0..15)
CB = OFF_FB1 + 256
OFF_MISC = CB                    # biases(12)+wcol(16) as bf16 hi then lo
CBX = CB + 56                    # total blobw cols
# node table rides in the int8 blob: nat [128,1024] cols f*64+hi, then
# snat [128,128] cols w*16+f; dequantized on device by nat_scale.
# vectors ride as int16 fixed-point split into hi/lo int8 planes.
NAT8 = 1024 + 128
VSCALE = 2.0 ** -14              # int16 fixed-point scale for vectors


def _host_shard(vectors, senders, receivers):
    """Group edges by (core, sender-window); pad to common K_WIN."""
    core = senders // NPC
    win = (senders % NPC) // WIN
    key = core * NW + win
    order = np.argsort(key, kind="stable")
    counts = np.bincount(key, minlength=NC * NW)
    kwin = int(((counts.max() + 127) // 128) * 128)
    starts = np.zeros(NC * NW + 1, np.int64)
    np.cumsum(counts, out=starts[1:])

    EP = NW * kwin
    shards = []
    for c in range(NC):
        v16 = np.zeros((EP, 3), np.int16)
        v16[:, 0] = 24576              # dummy edge: d = 1.5 -> u = 0
        sl = np.zeros(EP, np.int8)     # sender local-in-window
        rlo = np.zeros(EP, np.int8)
        rhi = np.zeros(EP, np.int8)
        for w in range(NW):
            g = c * NW + w
            eid = order[starts[g]:starts[g + 1]]
            o = w * kwin
            n_e = len(eid)
            v16[o:o + n_e] = np.round(vectors[eid] / VSCALE).astype(np.int16)
            sl[o:o + n_e] = (senders[eid] - (c * NPC + w * WIN)).astype(np.int8)
            rlo[o:o + n_e] = (receivers[eid] % 128).astype(np.int8)
            rhi[o:o + n_e] = (receivers[eid] // 128).astype(np.int8)
        shards.append((v16, sl, rlo, rhi))
    return kwin, shards


def _plane(a, T_ALL):
    """[EP] or [EP, k] -> plane layout [128, T_ALL*(k)] with e = t*128+p."""
    if a.ndim == 1:
        return np.ascontiguousarray(a.reshape(T_ALL, 128).T)
    # [EP, k] -> [128, k*T_ALL] with component-major column groups
    k = a.shape[1]
    p = a.reshape(T_ALL, 128, k).transpose(2, 1, 0)     # [k, 128, T_ALL]
    return np.ascontiguousarray(p.reshape(k * 128, T_ALL)).reshape(k, 128, T_ALL)


def _prep_weights(i):
    """Fold INV and residual 1/sqrt(2) scales into weights (f32)."""
    w = {}
    w["we0"] = i["W_e0"]                                       # [40,256]
    w["we1"] = i["W_e1"]
    w["wv0"] = i["W_v0"]
    w["wlw0"] = i["W_lw"][0] * INV
    w["wlw1"] = i["W_lw"][1] * INV * SQ                        # x1 = sq*x1'
    wly1_1 = i["W_ly1"][1].copy()
    wly1_1[:H] *= SQ                                           # x rows scaled
    w["wly1_0"] = i["W_ly1"][0]
    w["wly1_1"] = wly1_1
    w["wly2_0"] = i["W_ly2"][0]
    w["wly2_1"] = i["W_ly2"][1]
    w["wout"] = i["W_out"] * INV * 0.5                         # x2 = .5*x2'
    return w


def _pack_blobw(i):
    """Replicated bf16 weight blob [128, CB]."""
    w = _prep_weights(i)
    blob = np.zeros((128, CB), np.float32)
    # rhs row order is [snd attrs(16), rcv attrs(16), bessel(8)] so the
    # on-device copies land on legal partition offsets (0 and 32)
    blob[0:40, OFF_WE0:OFF_WE0 + 256] = np.vstack([w["we0"][8:40],
                                                   w["we0"][0:8]])
    blob[64:80, OFF_WE0:OFF_WE0 + 256] = w["wly1_0"][256:272]
    blob[0:16, OFF_FB1:OFF_FB1 + 256] = w["wly1_1"][256:272]
    for kc in range(2):
        s = slice(kc * 128, (kc + 1) * 128)
        blob[:, OFF_WE1 + kc * 256:OFF_WE1 + (kc + 1) * 256] = w["we1"][s]
        blob[:, OFF_WV0 + kc * 16:OFF_WV0 + (kc + 1) * 16] = w["wv0"][s]
        blob[:, OFF_WLW0 + kc * 16:OFF_WLW0 + (kc + 1) * 16] = w["wlw0"][s]
        blob[:, OFF_WLW1 + kc * 16:OFF_WLW1 + (kc + 1) * 16] = w["wlw1"][s]
        blob[:, OFF_WOUT + kc:OFF_WOUT + kc + 1] = w["wout"][s]
    for l in range(2):
        m = w[f"wly1_{l}"]
        for kc in range(2):
            s = slice(kc * 128, (kc + 1) * 128)
            blob[:, OFF_WLY1[l] + kc * 256:OFF_WLY1[l] + (kc + 1) * 256] = m[s]
            blob[:, OFF_WLY2[l] + kc * 256:OFF_WLY2[l] + (kc + 1) * 256] = \
                w[f"wly2_{l}"][s]
    return blob.astype(BF16)


def make_in_maps(inputs):
    kwin, shards = _host_shard(inputs["vectors"], inputs["senders"],
                               inputs["receivers"])
    EP = NW * kwin
    T_ALL = EP // 128
    bias_list = [inputs["b_e0"], inputs["b_e1"],
                 inputs["b_ly1"][0], inputs["b_ly1"][1],
                 inputs["b_ly2"][0], inputs["b_ly2"][1]]
    wcol = inputs["W_lsh"][0][:, 0]                            # [16]
    blobw = _pack_blobw(inputs)
    na = inputs["node_attrs"]                                  # [N, F]
    nat_scale = float(np.abs(na).max() / 127.0)
    naq = np.round(na / nat_scale).clip(-127, 127).astype(np.int8)
    # nat8[lo, f*64+hi] = naq[hi*128+lo, f]
    nat = naq.reshape(RWIN, 128, F).transpose(1, 2, 0).reshape(128, 1024)
    misc = np.zeros((128, 28), np.float32)
    for i, b in enumerate(bias_list):
        misc[:, 2 * i] = b[0:128]
        misc[:, 2 * i + 1] = b[128:256]
    misc[:, 12:28] = np.tile(wcol.reshape(1, 16), (128, 1))
    # f32 -> bf16 hi + bf16 lo pair (reconstructed by one add on device)
    mhi = misc.astype(BF16)
    mlo = (misc - mhi.astype(np.float32)).astype(BF16)
    blobx = np.zeros((128, CBX), BF16)
    blobx[:, 0:CB] = blobw
    blobx[:, OFF_MISC:OFF_MISC + 28] = mhi
    blobx[:, OFF_MISC + 28:OFF_MISC + 56] = mlo
    in_maps = []
    dbg = []
    for c in range(NC):
        v16, sl, rlo, rhi = shards[c]
        vhi = (v16 >> 8).astype(np.int8)
        vlo = ((v16 & 255) - 128).astype(np.int8)
        b8 = np.empty((128, 9 * T_ALL + NAT8), np.int8)
        b8[:, 0:T_ALL] = _plane(sl, T_ALL)
        b8[:, T_ALL:2 * T_ALL] = _plane(rlo, T_ALL)
        b8[:, 2 * T_ALL:3 * T_ALL] = _plane(rhi, T_ALL)
        b8[:, 3 * T_ALL:3 * T_ALL + 1024] = nat
        # snat[lo, w*16+f] = naq[(c*8+w)*128+lo, f]
        sn = naq.reshape(RWIN, 128, F)[c * NW:(c + 1) * NW]    # [w, lo, f]
        b8[:, 3 * T_ALL + 1024:3 * T_ALL + NAT8] = \
            sn.transpose(1, 0, 2).reshape(128, 128)
        o5 = 3 * T_ALL + NAT8
        o6 = o5 + 3 * T_ALL
        vph = _plane(vhi, T_ALL)                               # [3,128,T]
        vpl = _plane(vlo, T_ALL)
        for j in range(3):
            b8[:, o5 + j * T_ALL:o5 + (j + 1) * T_ALL] = vph[j]
            b8[:, o6 + j * T_ALL:o6 + (j + 1) * T_ALL] = vpl[j]
        in_maps.append({"blob8": b8, "blobw": blobx})
        dbg.append(dict(vec=v16.astype(np.float32) * VSCALE,
                        sl=sl, rlo=rlo, rhi=rhi))
    return kwin, nat_scale, in_maps, dbg


_CAP_SKIP = {"InstEventSemaphore", "InstBranch", "InstNop",
             "InstCollectiveCompute"}
_CAP_LIMITS = {}


def _split_waits(nc, mybir, mk_carrier, limit=1):
    """Walrus codegen allows only 1 embedded sem-wait on compute
    instructions.  For each instruction with more, strip the extras onto
    freshly created same-engine carrier instructions inserted directly
    before it (engines are in-order, so this preserves semantics)."""
    f = nc.m.functions[0]
    made = 0
    for bb in f.blocks:
        insts = list(bb.instructions)
        plan = []          # (index, [carrier insts])
        for i, inst in enumerate(insts):
            tname = type(inst).__name__
            si = inst.sync_info
            nwait = len(si.on_wait) if (si and si.on_wait) else 0
            lim = _CAP_LIMITS.get(tname, limit)
            if tname in _CAP_SKIP or nwait <= lim:
                continue
            waits = list(si.on_wait)
            extras, keep = waits[:-lim], waits[-lim:]
            carriers = []
            for wt in extras:
                ci = mk_carrier(inst.engine)
                if ci is None:
                    keep.insert(0, wt)
                    continue
                ci.sync_info = mybir.SyncInfo(on_wait=[wt], on_update=[])
                carriers.append(ci)
                made += 1
            inst.sync_info = mybir.SyncInfo(on_wait=keep,
                                            on_update=si.on_update)
            if carriers:
                plan.append((i, carriers))
        if plan:
            new = []
            pmap = dict(plan)
            for i, inst in enumerate(insts):
                if i in pmap:
                    new.extend(pmap[i])
                new.append(inst)
            bb.instructions = new
    return made


def build_graph(kwin, nat_scale):
    from concourse import bass, mybir
    from concourse.masks import make_identity
    from concourse.tile import TileContext

    EP = NW * kwin
    T_ALL = EP // 128
    T_W = kwin // 128
    NCH = (kwin + 511) // 512      # free chunks per window

    f32 = mybir.dt.float32
    bf16 = mybir.dt.bfloat16
    i32 = mybir.dt.int32
    i8 = mybir.dt.int8
    i16 = mybir.dt.int16
    AX = mybir.AxisListType.X
    OP = mybir.AluOpType
    AF = mybir.ActivationFunctionType

    nc = bass.Bass()
    carrier_sem_cm = nc.semaphore("carrier_sem")
    carrier_sem = carrier_sem_cm.__enter__()
    dp = nc.declare_dram_parameter
    d_b8 = dp("blob8", [128, 9 * T_ALL + NAT8], i8, isOutput=False)
    d_bw = dp("blobw", [128, CBX], bf16, isOutput=False)
    d_out = dp("out", [128, RWIN], f32, isOutput=True)

    with TileContext(nc) as tc:
        with (
            tc.tile_pool(name="glob", bufs=1) as gp,
            tc.tile_pool(name="wgt", bufs=1) as wp,
            tc.tile_pool(name="win", bufs=2) as wnp,
            tc.tile_pool(name="big", bufs=1) as bgp,
            tc.tile_pool(name="sml", bufs=3) as sp,
            tc.tile_pool(name="ps_mlp", bufs=2, space="PSUM") as pmlp,
            tc.tile_pool(name="ps_acc", bufs=1, space="PSUM") as pacc,
            tc.tile_pool(name="ps_gth", bufs=1, space="PSUM") as pgth,
            tc.tile_pool(name="ps_sml", bufs=1, space="PSUM") as psml,
            tc.tile_pool(name="ps_rcv", bufs=1, space="PSUM") as prcv,
        ):
            # ---------------- blobs to SBUF ----------------
            wb = wp.tile([128, CBX], bf16, tag="wb")
            nc.sync.dma_start(out=wb[:], in_=d_bw[:])
            i8t = wp.tile([128, 9 * T_ALL + NAT8], i8, tag="i8t")
            nc.sync.dma_start(out=i8t[:], in_=d_b8[:])
            # biases + wcol: f32 = bf16 hi + bf16 lo
            fbt = wp.tile([128, 28], f32, tag="fbt")
            nc.vector.tensor_add(out=fbt[:],
                                 in0=wb[:, OFF_MISC:OFF_MISC + 28],
                                 in1=wb[:, OFF_MISC + 28:OFF_MISC + 56])
            slf = wp.tile([128, T_ALL], f32, tag="slf")
            rlof = wp.tile([128, T_ALL], f32, tag="rlof")
            rhif = wp.tile([128, T_ALL], f32, tag="rhif")
            nc.vector.tensor_copy(out=slf[:], in_=i8t[:, 0:T_ALL])
            nc.vector.tensor_copy(out=rlof[:], in_=i8t[:, T_ALL:2 * T_ALL])
            nc.vector.tensor_copy(out=rhif[:], in_=i8t[:, 2 * T_ALL:3 * T_ALL])
            # dequantized node table (nat 1024 cols + snat 128 cols)
            natbf = wp.tile([128, NAT8], bf16, tag="natbf")
            nc.vector.tensor_scalar(
                out=natbf[:], in0=i8t[:, 3 * T_ALL:3 * T_ALL + NAT8],
                scalar1=float(nat_scale), scalar2=None, op0=OP.mult)

            ident = wp.tile([128, 128], f32, tag="ident")
            make_identity(nc, ident[:])
            identb = wp.tile([128, 128], bf16, tag="identb")
            make_identity(nc, identb[:])
            it32 = wp.tile([128, 128], i32, tag="it32")
            nc.gpsimd.iota(out=it32[:], pattern=[[1, 128]], base=0,
                           channel_multiplier=0)
            iof = wp.tile([128, 128], f32, tag="iof")
            nc.vector.tensor_copy(out=iof[:], in_=it32[:])
            ones_bf = wp.tile([1, 128], bf16, tag="ones")
            nc.gpsimd.memset(ones_bf[:], 1.0)

            # views into the blobs
            def bias(i, hc):
                return fbt[:, 2 * i + hc, None]
            wcol = fbt[:, 12:28]
            natv = natbf[:, 0:1024]

            # ---------------- edge-scalar stage (planes [128,T_ALL]) ----
            ta = gp.tile([128, T_ALL], f32)
            tb = gp.tile([128, T_ALL], f32)
            ts = nc.vector.tensor_scalar
            act = nc.scalar.activation
            # dequantize int16 fixed-point vectors from hi/lo int8 planes:
            # v = (hi*256 + lo + 128) * VSCALE
            vx = gp.tile([128, T_ALL], f32)
            vy = gp.tile([128, T_ALL], f32)
            vz = gp.tile([128, T_ALL], f32)
            o5 = 3 * T_ALL + NAT8
            o6 = o5 + 3 * T_ALL
            for j, vv in enumerate((vx, vy, vz)):
                ts(out=vv[:], in0=i8t[:, o5 + j * T_ALL:o5 + (j + 1) * T_ALL],
                   scalar1=256.0 * VSCALE, scalar2=None, op0=OP.mult)
                ts(out=ta[:], in0=i8t[:, o6 + j * T_ALL:o6 + (j + 1) * T_ALL],
                   scalar1=VSCALE, scalar2=128.0 * VSCALE,
                   op0=OP.mult, op1=OP.add)
                nc.vector.tensor_add(out=vv[:], in0=vv[:], in1=ta[:])

            def silu_act(out, ps_in, bias_ap):
                if not SIM_SILU:
                    act(out=out, in_=ps_in, func=AF.Silu, bias=bias_ap)
                else:
                    pp = ps_in.shape[0]
                    sg = bgp.tile([128, 512], f32, tag="simsilu")
                    zz_ = bgp.tile([128, 512], f32, tag="simsilu2")
                    cw_ = ps_in.shape[-1]
                    act(out=sg[:pp, :cw_], in_=ps_in, func=AF.Sigmoid,
                        bias=bias_ap)
                    nc.vector.tensor_scalar(out=zz_[:pp, :cw_], in0=ps_in,
                                            scalar1=bias_ap, scalar2=None,
                                            op0=OP.add)
                    nc.vector.tensor_mul(out=out, in0=sg[:pp, :cw_],
                                         in1=zz_[:pp, :cw_])
            d_pl = gp.tile([128, T_ALL], f32)
            nc.vector.tensor_mul(out=ta[:], in0=vx[:], in1=vx[:])
            nc.vector.tensor_mul(out=tb[:], in0=vy[:], in1=vy[:])
            nc.vector.tensor_add(out=ta[:], in0=ta[:], in1=tb[:])
            nc.vector.tensor_mul(out=tb[:], in0=vz[:], in1=vz[:])
            nc.vector.tensor_add(out=ta[:], in0=ta[:], in1=tb[:])
            act(out=d_pl[:], in_=ta[:], func=AF.Sqrt)
            rinv = gp.tile([128, T_ALL], f32)
            nc.vector.reciprocal(out=rinv[:], in_=d_pl[:])
            ux = gp.tile([128, T_ALL], f32)
            uy = gp.tile([128, T_ALL], f32)
            uz = gp.tile([128, T_ALL], f32)
            nc.vector.tensor_mul(out=ux[:], in0=vx[:], in1=rinv[:])
            nc.vector.tensor_mul(out=uy[:], in0=vy[:], in1=rinv[:])
            nc.vector.tensor_mul(out=uz[:], in0=vz[:], in1=rinv[:])

            # envelope u = 1 + d^6*(-28 + 48d - 21d^2), zero for d >= 1
            u_pl = gp.tile([128, T_ALL], f32)
            nc.vector.tensor_mul(out=ta[:], in0=d_pl[:], in1=d_pl[:])   # d2
            nc.vector.tensor_mul(out=tb[:], in0=ta[:], in1=d_pl[:])     # d3
            nc.vector.tensor_mul(out=tb[:], in0=tb[:], in1=tb[:])       # d6
            ts(out=ta[:], in0=ta[:], scalar1=-21.0, scalar2=None, op0=OP.mult)
            tc_q = gp.tile([128, T_ALL], f32)
            ts(out=tc_q[:], in0=d_pl[:], scalar1=48.0, scalar2=-28.0,
               op0=OP.mult, op1=OP.add)
            nc.vector.tensor_add(out=ta[:], in0=ta[:], in1=tc_q[:])
            nc.vector.tensor_mul(out=tb[:], in0=tb[:], in1=ta[:])
            ts(out=tb[:], in0=tb[:], scalar1=1.0, scalar2=None, op0=OP.add)
            ts(out=ta[:], in0=d_pl[:], scalar1=1.0, scalar2=None,
               op0=OP.is_lt)
            nc.vector.tensor_mul(out=u_pl[:], in0=tb[:], in1=ta[:])

            # spherical harmonics Y [128, T_ALL, 16] f32
            Yt = gp.tile([128, T_ALL, 16], f32)
            s3 = 3.0 ** 0.5; s5 = 5.0 ** 0.5; s15 = 15.0 ** 0.5
            s7 = 7.0 ** 0.5
            c33 = (35.0 / 8.0) ** 0.5; c32 = 105.0 ** 0.5
            c31 = (21.0 / 8.0) ** 0.5
            xx = gp.tile([128, T_ALL], f32)
            yy = gp.tile([128, T_ALL], f32)
            zz = gp.tile([128, T_ALL], f32)
            xy = gp.tile([128, T_ALL], f32)
            nc.vector.tensor_mul(out=xx[:], in0=ux[:], in1=ux[:])
            nc.vector.tensor_mul(out=yy[:], in0=uy[:], in1=uy[:])
            nc.vector.tensor_mul(out=zz[:], in0=uz[:], in1=uz[:])
            nc.vector.tensor_mul(out=xy[:], in0=ux[:], in1=uy[:])
            ts(out=Yt[:, :, 0], in0=ux[:], scalar1=0.0, scalar2=1.0,
               op0=OP.mult, op1=OP.add)
            ts(out=Yt[:, :, 1], in0=ux[:], scalar1=s3, scalar2=None,
               op0=OP.mult)
            ts(out=Yt[:, :, 2], in0=uy[:], scalar1=s3, scalar2=None,
               op0=OP.mult)
            ts(out=Yt[:, :, 3], in0=uz[:], scalar1=s3, scalar2=None,
               op0=OP.mult)
            ts(out=Yt[:, :, 4], in0=xy[:], scalar1=s15, scalar2=None,
               op0=OP.mult)
            nc.vector.tensor_mul(out=ta[:], in0=uy[:], in1=uz[:])
            ts(out=Yt[:, :, 5], in0=ta[:], scalar1=s15, scalar2=None,
               op0=OP.mult)
            ts(out=Yt[:, :, 6], in0=zz[:], scalar1=1.5 * s5,
               scalar2=-0.5 * s5, op0=OP.mult, op1=OP.add)
            nc.vector.tensor_mul(out=tb[:], in0=ux[:], in1=uz[:])
            ts(out=Yt[:, :, 7], in0=tb[:], scalar1=s15, scalar2=None,
               op0=OP.mult)
            xmy = gp.tile([128, T_ALL], f32)
            nc.vector.tensor_sub(out=xmy[:], in0=xx[:], in1=yy[:])
            ts(out=Yt[:, :, 8], in0=xmy[:], scalar1=0.5 * s15, scalar2=None,
               op0=OP.mult)
            ts(out=ta[:], in0=xx[:], scalar1=3.0, scalar2=None, op0=OP.mult)
            nc.vector.tensor_sub(out=ta[:], in0=ta[:], in1=yy[:])
            nc.vector.tensor_mul(out=ta[:], in0=ta[:], in1=uy[:])
            ts(out=Yt[:, :, 9], in0=ta[:], scalar1=c33, scalar2=None,
               op0=OP.mult)
            nc.vector.tensor_mul(out=ta[:], in0=xy[:], in1=uz[:])
            ts(out=Yt[:, :, 10], in0=ta[:], scalar1=c32, scalar2=None,
               op0=OP.mult)
            ts(out=ta[:], in0=zz[:], scalar1=5.0, scalar2=-1.0,
               op0=OP.mult, op1=OP.add)
            nc.vector.tensor_mul(out=tb[:], in0=ta[:], in1=uy[:])
            ts(out=Yt[:, :, 11], in0=tb[:], scalar1=c31, scalar2=None,
               op0=OP.mult)
            nc.vector.tensor_mul(out=tb[:], in0=ta[:], in1=ux[:])
            ts(out=Yt[:, :, 13], in0=tb[:], scalar1=c31, scalar2=None,
               op0=OP.mult)
            nc.vector.tensor_mul(out=ta[:], in0=zz[:], in1=uz[:])
            ts(out=ta[:], in0=ta[:], scalar1=2.5 * s7, scalar2=None,
               op0=OP.mult)
            ts(out=tb[:], in0=uz[:], scalar1=1.5 * s7, scalar2=None,
               op0=OP.mult)
            nc.vector.tensor_sub(out=Yt[:, :, 12], in0=ta[:], in1=tb[:])
            nc.vector.tensor_mul(out=ta[:], in0=xmy[:], in1=uz[:])
            ts(out=Yt[:, :, 14], in0=ta[:], scalar1=0.5 * c32, scalar2=None,
               op0=OP.mult)
            ts(out=ta[:], in0=yy[:], scalar1=3.0, scalar2=None, op0=OP.mult)
            nc.vector.tensor_sub(out=ta[:], in0=xx[:], in1=ta[:])
            nc.vector.tensor_mul(out=ta[:], in0=ta[:], in1=ux[:])
            ts(out=Yt[:, :, 15], in0=ta[:], scalar1=c33, scalar2=None,
               op0=OP.mult)

            # bessel (range-reduced): besu [128, T_ALL, 8]
            besu = gp.tile([128, T_ALL, 8], f32)
            rs = gp.tile([128, T_ALL], f32)
            ts(out=rs[:], in0=rinv[:], scalar1=math.sqrt(2.0), scalar2=None,
               op0=OP.mult)
            mi = gp.tile([128, T_ALL], mybir.dt.int32)
            for k in range(1, NB + 1):
                ts(out=ta[:], in0=d_pl[:], scalar1=0.5 * k, scalar2=None,
                   op0=OP.mult)
                nc.vector.tensor_copy(out=mi[:], in_=ta[:])
                nc.vector.tensor_copy(out=tb[:], in_=mi[:])
                nc.vector.tensor_sub(out=ta[:], in0=ta[:], in1=tb[:])
                # ta = frac in (-0.5, 1) whether the cast rounds or truncates
                ts(out=tb[:], in0=ta[:], scalar1=0.5, scalar2=None,
                   op0=OP.is_gt)
                nc.vector.tensor_sub(out=ta[:], in0=ta[:], in1=tb[:])
                act(out=ta[:], in_=ta[:], func=AF.Sin, scale=2.0 * math.pi)
                nc.vector.tensor_mul(out=besu[:, :, k - 1], in0=ta[:],
                                      in1=rs[:])

            # ---------------- persistent receiver accumulator ----------
            ps_rcv = prcv.tile([128, RWIN], f32, space="PSUM")

            # ---------------- window loop ----------------
            for w in range(NW):
                t0 = w * T_W
                ohs = wnp.tile([128, T_W, 128], bf16)   # [e, n]
                ohg = wnp.tile([128, T_W, 128], bf16)   # [n, e]
                rqs = wnp.tile([128, T_W, 128], bf16)   # [e, lo]
                rqg = wnp.tile([128, T_W, 128], bf16)   # [lo, e]
                rwt = wnp.tile([128, T_W, RWIN], bf16)  # [e, hi]
                xfm = wnp.tile([40, kwin], bf16)        # snd(16)+rcv(16)+bes(8)
                ufm = wnp.tile([1, kwin], bf16)
                tt = nc.vector.tensor_tensor
                tsw = slice(t0, t0 + T_W)
                tt(out=ohs[:],
                   in0=slf[:, tsw, None].to_broadcast([128, T_W, 128]),
                   in1=iof[:, None, :].to_broadcast([128, T_W, 128]),
                   op=OP.is_equal)
                tt(out=rqs[:],
                   in0=rlof[:, tsw, None].to_broadcast([128, T_W, 128]),
                   in1=iof[:, None, :].to_broadcast([128, T_W, 128]),
                   op=OP.is_equal)
                tt(out=rwt[:],
                   in0=rhif[:, tsw, None].to_broadcast([128, T_W, RWIN]),
                   in1=iof[:, None, 0:RWIN].to_broadcast([128, T_W, RWIN]),
                   op=OP.is_equal)
                for t in range(T_W):
                    tg = t0 + t
                    csl = slice(t * 128, (t + 1) * 128)
                    ptr = psml.tile([128, 128], bf16, space="PSUM", tag="trn")
                    nc.tensor.transpose(out=ptr[:], in_=ohs[:, t, :],
                                        identity=identb[:])
                    nc.vector.tensor_copy(out=ohg[:, t, :], in_=ptr[:])
                    ptr2 = psml.tile([128, 128], bf16, space="PSUM", tag="trn")
                    nc.tensor.transpose(out=ptr2[:], in_=rqs[:, t, :],
                                        identity=identb[:])
                    nc.vector.tensor_copy(out=rqg[:, t, :], in_=ptr2[:])
                    # bessel + u feature-major
                    pst = psml.tile([32, 128], f32, space="PSUM", tag="sml")
                    nc.tensor.transpose(out=pst[0:8, :], in_=besu[:, tg, :],
                                        identity=ident[:])
                    nc.vector.tensor_copy(out=xfm[32:40, csl],
                                          in_=pst[0:8, :])
                    psu1 = psml.tile([32, 128], f32, space="PSUM", tag="sml")
                    nc.tensor.transpose(out=psu1[0:1, :],
                                        in_=u_pl[:, tg, None],
                                        identity=ident[:])
                    nc.vector.tensor_copy(out=ufm[:, csl], in_=psu1[0:1, :])
                    # endpoint-attr gather: sender (window-local one-hot)
                    gcmb = sp.tile([128, 32], f32, tag="gcmb")
                    psn = psml.tile([128, 32], f32, space="PSUM", tag="sm2")
                    nc.tensor.matmul(
                        out=psn[:, 0:16], lhsT=ohg[:, t, :],
                        rhs=natbf[:, 1024 + w * 16:1024 + (w + 1) * 16],
                        start=True, stop=True)
                    nc.vector.tensor_copy(out=gcmb[:, 0:16], in_=psn[:, 0:16])
                    # receiver: lo-gather matmul then hi-select
                    for c2 in range(2):
                        prg = pgth.tile([128, 512], f32, space="PSUM",
                                        tag="gth")
                        nc.tensor.matmul(
                            out=prg[:], lhsT=rqg[:, t, :],
                            rhs=natv[:, c2 * 512:(c2 + 1) * 512],
                            start=True, stop=True)
                        prod = sp.tile([128, 8, RWIN], f32, tag="rsel")
                        nc.vector.tensor_mul(
                            out=prod[:],
                            in0=prg[:].rearrange("p (a b) -> p a b", b=RWIN),
                            in1=rwt[:, t, None, :].to_broadcast(
                                [128, 8, RWIN]))
                        nc.vector.reduce_sum(
                            out=gcmb[:, 16 + c2 * 8:16 + (c2 + 1) * 8, None],
                            in_=prod[:], axis=AX)
                    ptg = psml.tile([32, 128], f32, space="PSUM", tag="sml")
                    nc.tensor.transpose(out=ptg[:], in_=gcmb[:],
                                        identity=ident[:])
                    nc.vector.tensor_copy(out=xfm[0:32, csl], in_=ptg[:])

                # broadcast u row -> [128, kwin] bf16
                ubc = bgp.tile([128, kwin], bf16)
                for ch in range(NCH):
                    c0 = ch * 512
                    c1 = min(kwin, c0 + 512)
                    psu = pmlp.tile([128, 512], f32, space="PSUM", tag="mlp")
                    nc.tensor.matmul(out=psu[:, :c1 - c0], lhsT=ones_bf[:],
                                     rhs=ufm[:, c0:c1],
                                     start=True, stop=True)
                    nc.vector.tensor_copy(out=ubc[:, c0:c1],
                                          in_=psu[:, :c1 - c0])

                # ---- edge MLP: x0 = u*silu(e1(silu(e0(bes,attrs)))) ----
                x0 = bgp.tile([128, 2, kwin], bf16)
                th = bgp.tile([128, 2, kwin], bf16)
                for ch in range(NCH):
                    c0 = ch * 512
                    c1 = min(kwin, c0 + 512)
                    cw = c1 - c0
                    for hc in range(2):
                        ps = pmlp.tile([128, 512], f32, space="PSUM", tag="mlp")
                        nc.tensor.matmul(
                            out=ps[:, :cw],
                            lhsT=wb[0:40, OFF_WE0 + hc * 128:
                                    OFF_WE0 + (hc + 1) * 128],
                            rhs=xfm[:, c0:c1], start=True, stop=True)
                        silu_act(th[:, hc, c0:c1], ps[:, :cw], bias(0, hc))
                for ch in range(NCH):
                    c0 = ch * 512
                    c1 = min(kwin, c0 + 512)
                    cw = c1 - c0
                    for hc in range(2):
                        ps = pmlp.tile([128, 512], f32, space="PSUM", tag="mlp")
                        for kc in range(2):
                            nc.tensor.matmul(
                                out=ps[:, :cw],
                                lhsT=wb[:, OFF_WE1 + kc * 256 + hc * 128:
                                        OFF_WE1 + kc * 256 + (hc + 1) * 128],
                                rhs=th[:, kc, c0:c1],
                                start=(kc == 0), stop=(kc == 1))
                        silu_act(x0[:, hc, c0:c1], ps[:, :cw], bias(1, hc))
                for hc in range(2):
                    nc.vector.tensor_mul(out=x0[:, hc, :], in0=x0[:, hc, :],
                                          in1=ubc[:])

                # ---- xv, w0 (edge-major [128,16] per tile) ----
                xv = wnp.tile([128, T_W, MUL], f32)
                w0 = wnp.tile([128, T_W, MUL], bf16)
                for t in range(T_W):
                    tsl = slice(t * 128, (t + 1) * 128)
                    p12 = psml.tile([128, 32], f32, space="PSUM", tag="sm2")
                    for kc in range(2):
                        nc.tensor.matmul(
                            out=p12[:, 0:16], lhsT=x0[:, kc, tsl],
                            rhs=wb[:, OFF_WV0 + kc * 16:OFF_WV0 + (kc + 1) * 16],
                            start=(kc == 0), stop=(kc == 1))
                    for kc in range(2):
                        nc.tensor.matmul(
                            out=p12[:, 16:32], lhsT=x0[:, kc, tsl],
                            rhs=wb[:, OFF_WLW0 + kc * 16:
                                    OFF_WLW0 + (kc + 1) * 16],
                            start=(kc == 0), stop=(kc == 1))
                    nc.vector.tensor_copy(out=xv[:, t, :], in_=p12[:, 0:16])
                    nc.vector.tensor_copy(out=w0[:, t, :], in_=p12[:, 16:32])

                # ---- layer-0 scatter: wY[n, m*16+i] ----
                ps_acc = pacc.tile([128, 256], f32, space="PSUM", tag="acc")
                for t in range(T_W):
                    v2 = sp.tile([128, MUL, 16], bf16, tag="v2")
                    nc.vector.tensor_mul(
                        out=v2[:],
                        in0=w0[:, t, :, None].to_broadcast([128, MUL, 16]),
                        in1=Yt[:, t0 + t, None, :].to_broadcast(
                            [128, MUL, 16]))
                    nc.tensor.matmul(
                        out=ps_acc[:],
                        lhsT=ohs[:, t, :],
                        rhs=v2[:].rearrange("p a b -> p (a b)"),
                        start=(t == 0), stop=(t == T_W - 1))
                wY = wnp.tile([128, 256], bf16)
                nc.vector.tensor_copy(out=wY[:], in_=ps_acc[:])

                # ---- gather + Ytil contraction + feedback ----
                # fbfm lives at partitions 64..79 so its matmul shares the
                # base partition of the packed wly1fb_0 weights
                V10 = wnp.tile([128, T_W, MUL], f32)
                fbfm = wnp.tile([80, kwin], bf16)
                prod = wnp.tile([128, MUL, 16], f32)
                ytil = wnp.tile([128, MUL], f32)
                Ssb = wnp.tile([128, MUL], f32)
                fb = wnp.tile([128, MUL], f32)
                for t in range(T_W):
                    pgf = pgth.tile([128, 512], f32, space="PSUM", tag="gth")
                    pg = pgf[:, 0:256]
                    nc.tensor.matmul(out=pg, lhsT=ohg[:, t, :], rhs=wY[:],
                                     start=True, stop=True)
                    pg3 = pg.rearrange("p (a b) -> p a b", b=16)
                    nc.vector.tensor_mul(out=ytil[:], in0=Yt[:, t0 + t, :],
                                          in1=wcol)
                    nc.vector.tensor_mul(
                        out=prod[:], in0=pg3,
                        in1=ytil[:, None, :].to_broadcast([128, MUL, 16]))
                    nc.vector.reduce_sum(out=Ssb[:, :, None], in_=prod[:],
                                         axis=AX)
                    nc.vector.tensor_mul(out=V10[:, t, :], in0=Ssb[:],
                                          in1=xv[:, t, :])
                    nc.vector.tensor_mul(out=fb[:], in0=pg3[:, :, 0],
                                          in1=xv[:, t, :])
                    pst = psml.tile([32, 128], f32, space="PSUM", tag="sml")
                    nc.tensor.transpose(out=pst[0:16, :], in_=fb[:],
                                        identity=ident[:])
                    nc.vector.tensor_copy(
                        out=fbfm[64:80, t * 128:(t + 1) * 128],
                        in_=pst[0:16, :])

                # ---- layer-0 ly1/ly2 + residual -> x1 ----
                x1 = bgp.tile([128, 2, kwin], bf16)

                def mlp_block(xin, xout, l, fbrow, resid_sq2):
                    b1 = OFF_WLY1[l]
                    # wly1fb: layer 0 at rows 64..79 of the WE0 cols,
                    # layer 1 at rows 0..15 of its own FB1 cols
                    fbp, fbc = (64, OFF_WE0) if l == 0 else (0, OFF_FB1)
                    b2 = OFF_WLY2[l]
                    ty = bgp.tile([128, 2, kwin], bf16)
                    for ch in range(NCH):
                        c0 = ch * 512
                        c1 = min(kwin, c0 + 512)
                        cw = c1 - c0
                        for hc in range(2):
                            hs = slice(hc * 128, (hc + 1) * 128)
                            ps = pmlp.tile([128, 512], f32, space="PSUM",
                                           tag="mlp")
                            for kc in range(2):
                                nc.tensor.matmul(
                                    out=ps[:, :cw],
                                    lhsT=wb[:, b1 + kc * 256 + hc * 128:
                                            b1 + kc * 256 + (hc + 1) * 128],
                                    rhs=xin[:, kc, c0:c1],
                                    start=(kc == 0), stop=False)
                            nc.tensor.matmul(
                                out=ps[:, :cw],
                                lhsT=wb[fbp:fbp + 16, fbc + hc * 128:
                                        fbc + (hc + 1) * 128],
                                rhs=fbrow[fbp:fbp + 16, c0:c1],
                                start=False, stop=True)
                            silu_act(ty[:, hc, c0:c1], ps[:, :cw],
                                     bias(2 + l, hc))
                    ty2 = bgp.tile([128, 2, kwin], bf16)
                    for ch in range(NCH):
                        c0 = ch * 512
                        c1 = min(kwin, c0 + 512)
                        cw = c1 - c0
                        for hc in range(2):
                            ps = pmlp.tile([128, 512], f32, space="PSUM",
                                           tag="mlp")
                            for kc in range(2):
                                nc.tensor.matmul(
                                    out=ps[:, :cw],
                                    lhsT=wb[:, b2 + kc * 256 + hc * 128:
                                            b2 + kc * 256 + (hc + 1) * 128],
                                    rhs=ty[:, kc, c0:c1],
                                    start=(kc == 0), stop=(kc == 1))
                            silu_act(ty2[:, hc, c0:c1], ps[:, :cw],
                                     bias(4 + l, hc))
                    # x_out' = x_in' + s * u * y   (s = 1 or sqrt(2))
                    for hc in range(2):
                        nc.vector.tensor_mul(out=ty2[:, hc, :],
                                              in0=ty2[:, hc, :], in1=ubc[:])
                        if resid_sq2:
                            ts(out=ty2[:, hc, :], in0=ty2[:, hc, :],
                               scalar1=math.sqrt(2.0), scalar2=None,
                               op0=OP.mult)
                        nc.vector.tensor_add(out=xout[:, hc, :],
                                             in0=xin[:, hc, :],
                                             in1=ty2[:, hc, :])

                mlp_block(x0, x1, 0, fbfm, False)

                # ---- layer 1: w1, 16-wide scatter/gather, feedback ----
                w1 = wnp.tile([128, T_W, MUL], bf16)
                for t in range(T_W):
                    tsl = slice(t * 128, (t + 1) * 128)
                    p1 = psml.tile([128, 32], f32, space="PSUM", tag="sm2")
                    for kc in range(2):
                        nc.tensor.matmul(
                            out=p1[:, 0:MUL], lhsT=x1[:, kc, tsl],
                            rhs=wb[:, OFF_WLW1 + kc * 16:
                                    OFF_WLW1 + (kc + 1) * 16],
                            start=(kc == 0), stop=(kc == 1))
                    nc.vector.tensor_copy(out=w1[:, t, :], in_=p1[:, 0:MUL])
                ps_a1 = pacc.tile([128, 256], f32, space="PSUM", tag="acc")
                for t in range(T_W):
                    nc.tensor.matmul(out=ps_a1[:, 0:MUL], lhsT=ohs[:, t, :],
                                     rhs=w1[:, t, :],
                                     start=(t == 0), stop=(t == T_W - 1))
                wY1 = wnp.tile([128, MUL], bf16)
                nc.vector.tensor_copy(out=wY1[:], in_=ps_a1[:, 0:MUL])
                fbfm1 = wnp.tile([MUL, kwin], bf16)
                fb1 = wnp.tile([128, MUL], f32)
                for t in range(T_W):
                    pg = pgth.tile([128, 512], f32, space="PSUM", tag="gth")
                    nc.tensor.matmul(out=pg[:, 0:MUL], lhsT=ohg[:, t, :],
                                     rhs=wY1[:], start=True, stop=True)
                    nc.vector.tensor_mul(out=fb1[:], in0=pg[:, 0:MUL],
                                          in1=V10[:, t, :])
                    pst = psml.tile([32, 128], f32, space="PSUM", tag="sml")
                    nc.tensor.transpose(out=pst[0:16, :], in_=fb1[:],
                                        identity=ident[:])
                    nc.vector.tensor_copy(
                        out=fbfm1[:, t * 128:(t + 1) * 128],
                        in_=pst[0:16, :])

                # ---- layer-1 ly1/ly2 + residual -> x2 ----
                x2 = bgp.tile([128, 2, kwin], bf16)
                mlp_block(x1, x2, 1, fbfm1, True)

                # ---- edge out + receiver scatter ----
                eo = wnp.tile([128, 1], f32)
                mt = wnp.tile([128, RWIN], bf16)
                for t in range(T_W):
                    tsl = slice(t * 128, (t + 1) * 128)
                    p1 = psml.tile([128, 32], f32, space="PSUM", tag="sm2")
                    for kc in range(2):
                        nc.tensor.matmul(
                            out=p1[:, 0:1], lhsT=x2[:, kc, tsl],
                            rhs=wb[:, OFF_WOUT + kc:OFF_WOUT + kc + 1],
                            start=(kc == 0), stop=(kc == 1))
                    nc.vector.tensor_mul(out=eo[:], in0=p1[:, 0:1],
                                          in1=u_pl[:, t0 + t, None])
                    nc.vector.tensor_mul(
                        out=mt[:], in0=rwt[:, t, :],
                        in1=eo[:].to_broadcast([128, RWIN]))
                    nc.tensor.matmul(out=ps_rcv[:], lhsT=rqs[:, t, :],
                                     rhs=mt[:],
                                     start=(w == 0 and t == 0),
                                     stop=(w == NW - 1 and t == T_W - 1))

            out_sb = gp.tile([128, RWIN], f32)
            nc.vector.tensor_copy(out=out_sb[:], in_=ps_rcv[:])
            nc.sync.dma_start(out=d_out[:], in_=out_sb[:])

    ET = mybir.EngineType
    eng_map = {ET.DVE: nc.vector, ET.Activation: nc.scalar,
               ET.Pool: nc.gpsimd, ET.PE: nc.tensor, ET.SP: nc.sync}

    def mk_carrier(eng):
        be = eng_map.get(eng)
        if be is None:
            return None
        w = be.wait_ge(carrier_sem, 0)
        ci = w.ins if hasattr(w, "ins") else w
        for bb in nc.m.functions[0].blocks:
            il = list(bb.instructions)
            if any(x is ci for x in il):
                bb.instructions = [x for x in il if x is not ci]
                break
        return ci

    made = _split_waits(nc, mybir, mk_carrier)
    print(f"split_waits: carriers={made}", flush=True)
    return nc


def kernel(**inputs):
    inputs = {k: np.asarray(v) for k, v in inputs.items()}
    kwin, nat_scale, in_maps, _ = make_in_maps(inputs)
    nc = build_graph(kwin, nat_scale)
    from concourse.bass_utils import run_bass_kernel_spmd
    res = run_bass_kernel_spmd(nc, in_maps, core_ids=list(range(NC)))
    out = np.zeros((128, RWIN), np.float64)
    for r in res.results:
        out += np.asarray(r["out"], np.float64)
    # node n = hi*128 + lo stored at [lo, hi]
    return np.ascontiguousarray(out.T.reshape(N, 1)).astype(np.float32)


# revision 69
# speedup vs baseline: 1.1175x; 1.0215x over previous
"""Allegro-style GNN message passing on 8 TRN2 NeuronCores.

Strategy (v2 — minimal host->device bytes):
- Host: shard edges by SENDER node range (1024 nodes/core) -> sender
  segment-sums are fully core-local (no cross-core collectives).
  Within a core, group edges by 128-node sender windows; pad each
  (core, window) group to a common K_WIN with dummy edges (d=2 -> u=0 ->
  zero contribution).
- Inputs per core are just 3 packed blobs (~1.4 MB total): u8 index
  planes (sender-local / receiver-lo / receiver-hi), an f32 blob
  (edge vectors + biases + wcol), and a bf16 blob (node table +
  weights). One-hot scatter/gather matrices and endpoint-attribute
  gathers are built ON DEVICE (iota + is_equal + PE transposes +
  one-hot matmuls) instead of being shipped from the host -- the axon
  PJRT tunnel moves ~40 MB/s, so the previous 17.7 MB/core of host-
  built one-hots dominated wall time.
- Layer algebra: Y[:,0] == 1, so layer-1 only needs a 16-wide
  segment-sum of w1; W_lsh[1] output is dead; V1 is only needed at
  component 0 => contraction with Ytil = Y * W_lsh[0][:,0].
- Receiver scatter: node id = hi*128+lo; per edge-tile matmul with lo
  one-hot lhsT and (hi one-hot * edge_out) rhs accumulates [128,64]
  partials in PSUM; host sums the 8 per-core partials (the unshard).
- 1/sqrt(AVG_NEIGH) and the 1/sqrt(2) residual scales are folded into
  weights on the host.
"""
import math
import sys

import numpy as np

sys.path.insert(0, "/opt/trn_rl_repo")

import ml_dtypes  # noqa: E402

try:
    import jax
    jax.config.update("jax_compilation_cache_dir", "/tmp/jax_pcache")
    jax.config.update("jax_persistent_cache_min_entry_size_bytes", -1)
    jax.config.update("jax_persistent_cache_min_compile_time_secs", 0.0)
except Exception:
    pass

BF16 = ml_dtypes.bfloat16
SIM_SILU = False   # CoreSim lacks Silu; emulate with Sigmoid*z when set

N, E, MUL, H, F = 8192, 131072, 16, 256, 16
NB = 8
INV = 1.0 / math.sqrt(16.0)
NC = 8
NPC = N // NC          # nodes per core
WIN = 128
NW = NPC // WIN        # windows per core
RWIN = N // WIN        # 64 receiver windows
SQ = math.sqrt(0.5)

# ---- bf16 weight-blob column layout [128, CB] (replicated) ----
OFF_WE0 = 0                      # we0 [40, 256] rows 0..39; wly1fb_0
#   shares these cols at rows 64..79
OFF_WE1 = OFF_WE0 + 256          # we1 2 x [128, 256]
OFF_WV0 = OFF_WE1 + 512          # wv0 2 x [128, 16]
OFF_WLW0 = OFF_WV0 + 32
OFF_WLW1 = OFF_WLW0 + 32
OFF_WLY1 = (OFF_WLW1 + 32, OFF_WLW1 + 32 + 512)
OFF_WLY2 = (OFF_WLY1[1] + 512, OFF_WLY1[1] + 512 + 512)
OFF_WOUT = OFF_WLY2[1] + 512     # wout 2 x [128, 1]
OFF_FB1 = OFF_WOUT + 2           # wly1fb_1 [16, 256] (rows 0..15)
CB = OFF_FB1 + 256
OFF_MISC = CB                    # biases(12)+wcol(16) as bf16 hi then lo
CBX = CB + 56                    # total blobw cols
# node table rides in the int8 blob: nat [128,1024] cols f*64+hi, then
# snat [128,128] cols w*16+f; dequantized on device by nat_scale.
# vectors ride as int16 fixed-point split into hi/lo int8 planes.
NAT8 = 1024 + 128
VSCALE = 2.0 ** -14              # int16 fixed-point scale for vectors


def _host_shard(vectors, senders, receivers):
    """Group edges by (core, sender-window); pad to common K_WIN."""
    core = senders // NPC
    win = (senders % NPC) // WIN
    key = core * NW + win
    order = np.argsort(key, kind="stable")
    counts = np.bincount(key, minlength=NC * NW)
    kwin = int(((counts.max() + 127) // 128) * 128)
    starts = np.zeros(NC * NW + 1, np.int64)
    np.cumsum(counts, out=starts[1:])

    EP = NW * kwin
    shards = []
    for c in range(NC):
        v16 = np.zeros((EP, 3), np.int16)
        v16[:, 0] = 24576              # dummy edge: d = 1.5 -> u = 0
        sl = np.zeros(EP, np.int8)     # sender local-in-window
        rlo = np.zeros(EP, np.int8)
        rhi = np.zeros(EP, np.int8)
        for w in range(NW):
            g = c * NW + w
            eid = order[starts[g]:starts[g + 1]]
            o = w * kwin
            n_e = len(eid)
            v16[o:o + n_e] = np.round(vectors[eid] / VSCALE).astype(np.int16)
            sl[o:o + n_e] = (senders[eid] - (c * NPC + w * WIN)).astype(np.int8)
            rlo[o:o + n_e] = (receivers[eid] % 128).astype(np.int8)
            rhi[o:o + n_e] = (receivers[eid] // 128).astype(np.int8)
        shards.append((v16, sl, rlo, rhi))
    return kwin, shards


def _plane(a, T_ALL):
    """[EP] or [EP, k] -> plane layout [128, T_ALL*(k)] with e = t*128+p."""
    if a.ndim == 1:
        return np.ascontiguousarray(a.reshape(T_ALL, 128).T)
    # [EP, k] -> [128, k*T_ALL] with component-major column groups
    k = a.shape[1]
    p = a.reshape(T_ALL, 128, k).transpose(2, 1, 0)     # [k, 128, T_ALL]
    return np.ascontiguousarray(p.reshape(k * 128, T_ALL)).reshape(k, 128, T_ALL)


def _prep_weights(i):
    """Fold INV and residual 1/sqrt(2) scales into weights (f32)."""
    w = {}
    w["we0"] = i["W_e0"]                                       # [40,256]
    w["we1"] = i["W_e1"]
    w["wv0"] = i["W_v0"]
    w["wlw0"] = i["W_lw"][0] * INV
    w["wlw1"] = i["W_lw"][1] * INV * SQ                        # x1 = sq*x1'
    wly1_1 = i["W_ly1"][1].copy()
    wly1_1[:H] *= SQ                                           # x rows scaled
    w["wly1_0"] = i["W_ly1"][0]
    w["wly1_1"] = wly1_1
    w["wly2_0"] = i["W_ly2"][0]
    w["wly2_1"] = i["W_ly2"][1]
    w["wout"] = i["W_out"] * INV * 0.5                         # x2 = .5*x2'
    return w


def _pack_blobw(i):
    """Replicated bf16 weight blob [128, CB]."""
    w = _prep_weights(i)
    blob = np.zeros((128, CB), np.float32)
    # rhs row order is [snd attrs(16), rcv attrs(16), bessel(8)] so the
    # on-device copies land on legal partition offsets (0 and 32)
    blob[0:40, OFF_WE0:OFF_WE0 + 256] = np.vstack([w["we0"][8:40],
                                                   w["we0"][0:8]])
    blob[64:80, OFF_WE0:OFF_WE0 + 256] = w["wly1_0"][256:272]
    blob[0:16, OFF_FB1:OFF_FB1 + 256] = w["wly1_1"][256:272]
    for kc in range(2):
        s = slice(kc * 128, (kc + 1) * 128)
        blob[:, OFF_WE1 + kc * 256:OFF_WE1 + (kc + 1) * 256] = w["we1"][s]
        blob[:, OFF_WV0 + kc * 16:OFF_WV0 + (kc + 1) * 16] = w["wv0"][s]
        blob[:, OFF_WLW0 + kc * 16:OFF_WLW0 + (kc + 1) * 16] = w["wlw0"][s]
        blob[:, OFF_WLW1 + kc * 16:OFF_WLW1 + (kc + 1) * 16] = w["wlw1"][s]
        blob[:, OFF_WOUT + kc:OFF_WOUT + kc + 1] = w["wout"][s]
    for l in range(2):
        m = w[f"wly1_{l}"]
        for kc in range(2):
            s = slice(kc * 128, (kc + 1) * 128)
            blob[:, OFF_WLY1[l] + kc * 256:OFF_WLY1[l] + (kc + 1) * 256] = m[s]
            blob[:, OFF_WLY2[l] + kc * 256:OFF_WLY2[l] + (kc + 1) * 256] = \
                w[f"wly2_{l}"][s]
    return blob.astype(BF16)


def make_in_maps(inputs):
    kwin, shards = _host_shard(inputs["vectors"], inputs["senders"],
                               inputs["receivers"])
    EP = NW * kwin
    T_ALL = EP // 128
    bias_list = [inputs["b_e0"], inputs["b_e1"],
                 inputs["b_ly1"][0], inputs["b_ly1"][1],
                 inputs["b_ly2"][0], inputs["b_ly2"][1]]
    wcol = inputs["W_lsh"][0][:, 0]                            # [16]
    blobw = _pack_blobw(inputs)
    na = inputs["node_attrs"]                                  # [N, F]
    nat_scale = float(np.abs(na).max() / 127.0)
    naq = np.round(na / nat_scale).clip(-127, 127).astype(np.int8)
    # nat8[lo, f*64+hi] = naq[hi*128+lo, f]
    nat = naq.reshape(RWIN, 128, F).transpose(1, 2, 0).reshape(128, 1024)
    misc = np.zeros((128, 28), np.float32)
    for i, b in enumerate(bias_list):
        misc[:, 2 * i] = b[0:128]
        misc[:, 2 * i + 1] = b[128:256]
    misc[:, 12:28] = np.tile(wcol.reshape(1, 16), (128, 1))
    # f32 -> bf16 hi + bf16 lo pair (reconstructed by one add on device)
    mhi = misc.astype(BF16)
    mlo = (misc - mhi.astype(np.float32)).astype(BF16)
    blobx = np.zeros((128, CBX), BF16)
    blobx[:, 0:CB] = blobw
    blobx[:, OFF_MISC:OFF_MISC + 28] = mhi
    blobx[:, OFF_MISC + 28:OFF_MISC + 56] = mlo
    # ship as separate hi/lo BYTE planes: the exponent-byte plane
    # compresses ~2x better on the wire than interleaved bf16
    w16 = blobx.view(np.int16)
    bw8 = np.empty((128, 2 * CBX), np.int8)
    bw8[:, 0:CBX] = (w16 >> 8).astype(np.int8)
    bw8[:, CBX:2 * CBX] = ((w16 & 255) - 128).astype(np.int8)
    in_maps = []
    dbg = []
    for c in range(NC):
        v16, sl, rlo, rhi = shards[c]
        vhi = (v16 >> 8).astype(np.int8)
        vlo = ((v16 & 255) - 128).astype(np.int8)
        b8 = np.empty((128, 9 * T_ALL + NAT8), np.int8)
        b8[:, 0:T_ALL] = _plane(sl, T_ALL)
        b8[:, T_ALL:2 * T_ALL] = _plane(rlo, T_ALL)
        b8[:, 2 * T_ALL:3 * T_ALL] = _plane(rhi, T_ALL)
        b8[:, 3 * T_ALL:3 * T_ALL + 1024] = nat
        # snat[lo, w*16+f] = naq[(c*8+w)*128+lo, f]
        sn = naq.reshape(RWIN, 128, F)[c * NW:(c + 1) * NW]    # [w, lo, f]
        b8[:, 3 * T_ALL + 1024:3 * T_ALL + NAT8] = \
            sn.transpose(1, 0, 2).reshape(128, 128)
        o5 = 3 * T_ALL + NAT8
        o6 = o5 + 3 * T_ALL
        vph = _plane(vhi, T_ALL)                               # [3,128,T]
        vpl = _plane(vlo, T_ALL)
        for j in range(3):
            b8[:, o5 + j * T_ALL:o5 + (j + 1) * T_ALL] = vph[j]
            b8[:, o6 + j * T_ALL:o6 + (j + 1) * T_ALL] = vpl[j]
        in_maps.append({"blob8": b8, "blobw8": bw8})
        dbg.append(dict(vec=v16.astype(np.float32) * VSCALE,
                        sl=sl, rlo=rlo, rhi=rhi))
    return kwin, nat_scale, in_maps, dbg


_CAP_SKIP = {"InstEventSemaphore", "InstBranch", "InstNop",
             "InstCollectiveCompute"}
_CAP_LIMITS = {}


def _split_waits(nc, mybir, mk_carrier, limit=1):
    """Walrus codegen allows only 1 embedded sem-wait on compute
    instructions.  For each instruction with more, strip the extras onto
    freshly created same-engine carrier instructions inserted directly
    before it (engines are in-order, so this preserves semantics)."""
    f = nc.m.functions[0]
    made = 0
    for bb in f.blocks:
        insts = list(bb.instructions)
        plan = []          # (index, [carrier insts])
        for i, inst in enumerate(insts):
            tname = type(inst).__name__
            si = inst.sync_info
            nwait = len(si.on_wait) if (si and si.on_wait) else 0
            lim = _CAP_LIMITS.get(tname, limit)
            if tname in _CAP_SKIP or nwait <= lim:
                continue
            waits = list(si.on_wait)
            extras, keep = waits[:-lim], waits[-lim:]
            carriers = []
            for wt in extras:
                ci = mk_carrier(inst.engine)
                if ci is None:
                    keep.insert(0, wt)
                    continue
                ci.sync_info = mybir.SyncInfo(on_wait=[wt], on_update=[])
                carriers.append(ci)
                made += 1
            inst.sync_info = mybir.SyncInfo(on_wait=keep,
                                            on_update=si.on_update)
            if carriers:
                plan.append((i, carriers))
        if plan:
            new = []
            pmap = dict(plan)
            for i, inst in enumerate(insts):
                if i in pmap:
                    new.extend(pmap[i])
                new.append(inst)
            bb.instructions = new
    return made


def build_graph(kwin, nat_scale):
    from concourse import bass, mybir
    from concourse.masks import make_identity
    from concourse.tile import TileContext

    EP = NW * kwin
    T_ALL = EP // 128
    T_W = kwin // 128
    NCH = (kwin + 511) // 512      # free chunks per window

    f32 = mybir.dt.float32
    bf16 = mybir.dt.bfloat16
    i32 = mybir.dt.int32
    i8 = mybir.dt.int8
    i16 = mybir.dt.int16
    AX = mybir.AxisListType.X
    OP = mybir.AluOpType
    AF = mybir.ActivationFunctionType

    nc = bass.Bass()
    carrier_sem_cm = nc.semaphore("carrier_sem")
    carrier_sem = carrier_sem_cm.__enter__()
    dp = nc.declare_dram_parameter
    d_b8 = dp("blob8", [128, 9 * T_ALL + NAT8], i8, isOutput=False)
    d_bw = dp("blobw8", [128, 2 * CBX], i8, isOutput=False)
    d_out = dp("out", [128, RWIN], f32, isOutput=True)

    with TileContext(nc) as tc:
        with (
            tc.tile_pool(name="glob", bufs=1) as gp,
            tc.tile_pool(name="wgt", bufs=1) as wp,
            tc.tile_pool(name="win", bufs=2) as wnp,
            tc.tile_pool(name="big", bufs=1) as bgp,
            tc.tile_pool(name="sml", bufs=3) as sp,
            tc.tile_pool(name="ps_mlp", bufs=2, space="PSUM") as pmlp,
            tc.tile_pool(name="ps_acc", bufs=1, space="PSUM") as pacc,
            tc.tile_pool(name="ps_gth", bufs=1, space="PSUM") as pgth,
            tc.tile_pool(name="ps_sml", bufs=1, space="PSUM") as psml,
            tc.tile_pool(name="ps_rcv", bufs=1, space="PSUM") as prcv,
        ):
            # ---------------- blobs to SBUF ----------------
            # reassemble bf16 weight bits from hi/lo byte planes:
            # i16 value = hi*256 + (lo+128), then bitcast to bf16
            i8w = wp.tile([128, 2 * CBX], i8, tag="i8w")
            nc.sync.dma_start(out=i8w[:], in_=d_bw[:])
            wbi16 = wp.tile([128, CBX], i16, tag="wbi16")
            wtmp = wp.tile([128, 512], f32, tag="wtmp")
            for c0 in range(0, CBX, 512):
                c1 = min(CBX, c0 + 512)
                cw_ = c1 - c0
                nc.vector.tensor_scalar(
                    out=wtmp[:, 0:cw_], in0=i8w[:, CBX + c0:CBX + c1],
                    scalar1=1.0, scalar2=128.0, op0=OP.mult, op1=OP.add)
                nc.vector.scalar_tensor_tensor(
                    out=wbi16[:, c0:c1], in0=i8w[:, c0:c1], scalar=256.0,
                    in1=wtmp[:, 0:cw_], op0=OP.mult, op1=OP.add)
            wb = wp.tile([128, CBX], bf16, tag="wb")
            nc.vector.tensor_copy(out=wb[:], in_=wbi16[:].bitcast(bf16))
            i8t = wp.tile([128, 9 * T_ALL + NAT8], i8, tag="i8t")
            nc.sync.dma_start(out=i8t[:], in_=d_b8[:])
            # biases + wcol: f32 = bf16 hi + bf16 lo
            fbt = wp.tile([128, 28], f32, tag="fbt")
            nc.vector.tensor_add(out=fbt[:],
                                 in0=wb[:, OFF_MISC:OFF_MISC + 28],
                                 in1=wb[:, OFF_MISC + 28:OFF_MISC + 56])
            slf = wp.tile([128, T_ALL], f32, tag="slf")
            rlof = wp.tile([128, T_ALL], f32, tag="rlof")
            rhif = wp.tile([128, T_ALL], f32, tag="rhif")
            nc.vector.tensor_copy(out=slf[:], in_=i8t[:, 0:T_ALL])
            nc.vector.tensor_copy(out=rlof[:], in_=i8t[:, T_ALL:2 * T_ALL])
            nc.vector.tensor_copy(out=rhif[:], in_=i8t[:, 2 * T_ALL:3 * T_ALL])
            # dequantized node table (nat 1024 cols + snat 128 cols)
            natbf = wp.tile([128, NAT8], bf16, tag="natbf")
            nc.vector.tensor_scalar(
                out=natbf[:], in0=i8t[:, 3 * T_ALL:3 * T_ALL + NAT8],
                scalar1=float(nat_scale), scalar2=None, op0=OP.mult)

            ident = wp.tile([128, 128], f32, tag="ident")
            make_identity(nc, ident[:])
            identb = wp.tile([128, 128], bf16, tag="identb")
            make_identity(nc, identb[:])
            it32 = wp.tile([128, 128], i32, tag="it32")
            nc.gpsimd.iota(out=it32[:], pattern=[[1, 128]], base=0,
                           channel_multiplier=0)
            iof = wp.tile([128, 128], f32, tag="iof")
            nc.vector.tensor_copy(out=iof[:], in_=it32[:])
            ones_bf = wp.tile([1, 128], bf16, tag="ones")
            nc.gpsimd.memset(ones_bf[:], 1.0)

            # views into the blobs
            def bias(i, hc):
                return fbt[:, 2 * i + hc, None]
            wcol = fbt[:, 12:28]
            natv = natbf[:, 0:1024]

            # ---------------- edge-scalar stage (planes [128,T_ALL]) ----
            ta = gp.tile([128, T_ALL], f32)
            tb = gp.tile([128, T_ALL], f32)
            ts = nc.vector.tensor_scalar
            act = nc.scalar.activation
            # dequantize int16 fixed-point vectors from hi/lo int8 planes:
            # v = (hi*256 + lo + 128) * VSCALE
            vx = gp.tile([128, T_ALL], f32)
            vy = gp.tile([128, T_ALL], f32)
            vz = gp.tile([128, T_ALL], f32)
            o5 = 3 * T_ALL + NAT8
            o6 = o5 + 3 * T_ALL
            for j, vv in enumerate((vx, vy, vz)):
                ts(out=vv[:], in0=i8t[:, o5 + j * T_ALL:o5 + (j + 1) * T_ALL],
                   scalar1=256.0 * VSCALE, scalar2=None, op0=OP.mult)
                ts(out=ta[:], in0=i8t[:, o6 + j * T_ALL:o6 + (j + 1) * T_ALL],
                   scalar1=VSCALE, scalar2=128.0 * VSCALE,
                   op0=OP.mult, op1=OP.add)
                nc.vector.tensor_add(out=vv[:], in0=vv[:], in1=ta[:])

            def silu_act(out, ps_in, bias_ap):
                if not SIM_SILU:
                    act(out=out, in_=ps_in, func=AF.Silu, bias=bias_ap)
                else:
                    pp = ps_in.shape[0]
                    sg = bgp.tile([128, 512], f32, tag="simsilu")
                    zz_ = bgp.tile([128, 512], f32, tag="simsilu2")
                    cw_ = ps_in.shape[-1]
                    act(out=sg[:pp, :cw_], in_=ps_in, func=AF.Sigmoid,
                        bias=bias_ap)
                    nc.vector.tensor_scalar(out=zz_[:pp, :cw_], in0=ps_in,
                                            scalar1=bias_ap, scalar2=None,
                                            op0=OP.add)
                    nc.vector.tensor_mul(out=out, in0=sg[:pp, :cw_],
                                         in1=zz_[:pp, :cw_])
            d_pl = gp.tile([128, T_ALL], f32)
            nc.vector.tensor_mul(out=ta[:], in0=vx[:], in1=vx[:])
            nc.vector.tensor_mul(out=tb[:], in0=vy[:], in1=vy[:])
            nc.vector.tensor_add(out=ta[:], in0=ta[:], in1=tb[:])
            nc.vector.tensor_mul(out=tb[:], in0=vz[:], in1=vz[:])
            nc.vector.tensor_add(out=ta[:], in0=ta[:], in1=tb[:])
            act(out=d_pl[:], in_=ta[:], func=AF.Sqrt)
            rinv = gp.tile([128, T_ALL], f32)
            nc.vector.reciprocal(out=rinv[:], in_=d_pl[:])
            ux = gp.tile([128, T_ALL], f32)
            uy = gp.tile([128, T_ALL], f32)
            uz = gp.tile([128, T_ALL], f32)
            nc.vector.tensor_mul(out=ux[:], in0=vx[:], in1=rinv[:])
            nc.vector.tensor_mul(out=uy[:], in0=vy[:], in1=rinv[:])
            nc.vector.tensor_mul(out=uz[:], in0=vz[:], in1=rinv[:])

            # envelope u = 1 + d^6*(-28 + 48d - 21d^2), zero for d >= 1
            u_pl = gp.tile([128, T_ALL], f32)
            nc.vector.tensor_mul(out=ta[:], in0=d_pl[:], in1=d_pl[:])   # d2
            nc.vector.tensor_mul(out=tb[:], in0=ta[:], in1=d_pl[:])     # d3
            nc.vector.tensor_mul(out=tb[:], in0=tb[:], in1=tb[:])       # d6
            ts(out=ta[:], in0=ta[:], scalar1=-21.0, scalar2=None, op0=OP.mult)
            tc_q = gp.tile([128, T_ALL], f32)
            ts(out=tc_q[:], in0=d_pl[:], scalar1=48.0, scalar2=-28.0,
               op0=OP.mult, op1=OP.add)
            nc.vector.tensor_add(out=ta[:], in0=ta[:], in1=tc_q[:])
            nc.vector.tensor_mul(out=tb[:], in0=tb[:], in1=ta[:])
            ts(out=tb[:], in0=tb[:], scalar1=1.0, scalar2=None, op0=OP.add)
            ts(out=ta[:], in0=d_pl[:], scalar1=1.0, scalar2=None,
               op0=OP.is_lt)
            nc.vector.tensor_mul(out=u_pl[:], in0=tb[:], in1=ta[:])

            # spherical harmonics Y [128, T_ALL, 16] f32
            Yt = gp.tile([128, T_ALL, 16], f32)
            s3 = 3.0 ** 0.5; s5 = 5.0 ** 0.5; s15 = 15.0 ** 0.5
            s7 = 7.0 ** 0.5
            c33 = (35.0 / 8.0) ** 0.5; c32 = 105.0 ** 0.5
            c31 = (21.0 / 8.0) ** 0.5
            xx = gp.tile([128, T_ALL], f32)
            yy = gp.tile([128, T_ALL], f32)
            zz = gp.tile([128, T_ALL], f32)
            xy = gp.tile([128, T_ALL], f32)
            nc.vector.tensor_mul(out=xx[:], in0=ux[:], in1=ux[:])
            nc.vector.tensor_mul(out=yy[:], in0=uy[:], in1=uy[:])
            nc.vector.tensor_mul(out=zz[:], in0=uz[:], in1=uz[:])
            nc.vector.tensor_mul(out=xy[:], in0=ux[:], in1=uy[:])
            ts(out=Yt[:, :, 0], in0=ux[:], scalar1=0.0, scalar2=1.0,
               op0=OP.mult, op1=OP.add)
            ts(out=Yt[:, :, 1], in0=ux[:], scalar1=s3, scalar2=None,
               op0=OP.mult)
            ts(out=Yt[:, :, 2], in0=uy[:], scalar1=s3, scalar2=None,
               op0=OP.mult)
            ts(out=Yt[:, :, 3], in0=uz[:], scalar1=s3, scalar2=None,
               op0=OP.mult)
            ts(out=Yt[:, :, 4], in0=xy[:], scalar1=s15, scalar2=None,
               op0=OP.mult)
            nc.vector.tensor_mul(out=ta[:], in0=uy[:], in1=uz[:])
            ts(out=Yt[:, :, 5], in0=ta[:], scalar1=s15, scalar2=None,
               op0=OP.mult)
            ts(out=Yt[:, :, 6], in0=zz[:], scalar1=1.5 * s5,
               scalar2=-0.5 * s5, op0=OP.mult, op1=OP.add)
            nc.vector.tensor_mul(out=tb[:], in0=ux[:], in1=uz[:])
            ts(out=Yt[:, :, 7], in0=tb[:], scalar1=s15, scalar2=None,
               op0=OP.mult)
            xmy = gp.tile([128, T_ALL], f32)
            nc.vector.tensor_sub(out=xmy[:], in0=xx[:], in1=yy[:])
            ts(out=Yt[:, :, 8], in0=xmy[:], scalar1=0.5 * s15, scalar2=None,
               op0=OP.mult)
            ts(out=ta[:], in0=xx[:], scalar1=3.0, scalar2=None, op0=OP.mult)
            nc.vector.tensor_sub(out=ta[:], in0=ta[:], in1=yy[:])
            nc.vector.tensor_mul(out=ta[:], in0=ta[:], in1=uy[:])
            ts(out=Yt[:, :, 9], in0=ta[:], scalar1=c33, scalar2=None,
               op0=OP.mult)
            nc.vector.tensor_mul(out=ta[:], in0=xy[:], in1=uz[:])
            ts(out=Yt[:, :, 10], in0=ta[:], scalar1=c32, scalar2=None,
               op0=OP.mult)
            ts(out=ta[:], in0=zz[:], scalar1=5.0, scalar2=-1.0,
               op0=OP.mult, op1=OP.add)
            nc.vector.tensor_mul(out=tb[:], in0=ta[:], in1=uy[:])
            ts(out=Yt[:, :, 11], in0=tb[:], scalar1=c31, scalar2=None,
               op0=OP.mult)
            nc.vector.tensor_mul(out=tb[:], in0=ta[:], in1=ux[:])
            ts(out=Yt[:, :, 13], in0=tb[:], scalar1=c31, scalar2=None,
               op0=OP.mult)
            nc.vector.tensor_mul(out=ta[:], in0=zz[:], in1=uz[:])
            ts(out=ta[:], in0=ta[:], scalar1=2.5 * s7, scalar2=None,
               op0=OP.mult)
            ts(out=tb[:], in0=uz[:], scalar1=1.5 * s7, scalar2=None,
               op0=OP.mult)
            nc.vector.tensor_sub(out=Yt[:, :, 12], in0=ta[:], in1=tb[:])
            nc.vector.tensor_mul(out=ta[:], in0=xmy[:], in1=uz[:])
            ts(out=Yt[:, :, 14], in0=ta[:], scalar1=0.5 * c32, scalar2=None,
               op0=OP.mult)
            ts(out=ta[:], in0=yy[:], scalar1=3.0, scalar2=None, op0=OP.mult)
            nc.vector.tensor_sub(out=ta[:], in0=xx[:], in1=ta[:])
            nc.vector.tensor_mul(out=ta[:], in0=ta[:], in1=ux[:])
            ts(out=Yt[:, :, 15], in0=ta[:], scalar1=c33, scalar2=None,
               op0=OP.mult)

            # bessel (range-reduced): besu [128, T_ALL, 8]
            besu = gp.tile([128, T_ALL, 8], f32)
            rs = gp.tile([128, T_ALL], f32)
            ts(out=rs[:], in0=rinv[:], scalar1=math.sqrt(2.0), scalar2=None,
               op0=OP.mult)
            mi = gp.tile([128, T_ALL], mybir.dt.int32)
            for k in range(1, NB + 1):
                ts(out=ta[:], in0=d_pl[:], scalar1=0.5 * k, scalar2=None,
                   op0=OP.mult)
                nc.vector.tensor_copy(out=mi[:], in_=ta[:])
                nc.vector.tensor_copy(out=tb[:], in_=mi[:])
                nc.vector.tensor_sub(out=ta[:], in0=ta[:], in1=tb[:])
                # ta = frac in (-0.5, 1) whether the cast rounds or truncates
                ts(out=tb[:], in0=ta[:], scalar1=0.5, scalar2=None,
                   op0=OP.is_gt)
                nc.vector.tensor_sub(out=ta[:], in0=ta[:], in1=tb[:])
                act(out=ta[:], in_=ta[:], func=AF.Sin, scale=2.0 * math.pi)
                nc.vector.tensor_mul(out=besu[:, :, k - 1], in0=ta[:],
                                      in1=rs[:])

            # ---------------- persistent receiver accumulator ----------
            ps_rcv = prcv.tile([128, RWIN], f32, space="PSUM")

            # ---------------- window loop ----------------
            for w in range(NW):
                t0 = w * T_W
                ohs = wnp.tile([128, T_W, 128], bf16)   # [e, n]
                ohg = wnp.tile([128, T_W, 128], bf16)   # [n, e]
                rqs = wnp.tile([128, T_W, 128], bf16)   # [e, lo]
                rqg = wnp.tile([128, T_W, 128], bf16)   # [lo, e]
                rwt = wnp.tile([128, T_W, RWIN], bf16)  # [e, hi]
                xfm = wnp.tile([40, kwin], bf16)        # snd(16)+rcv(16)+bes(8)
                ufm = wnp.tile([1, kwin], bf16)
                tt = nc.vector.tensor_tensor
                tsw = slice(t0, t0 + T_W)
                tt(out=ohs[:],
                   in0=slf[:, tsw, None].to_broadcast([128, T_W, 128]),
                   in1=iof[:, None, :].to_broadcast([128, T_W, 128]),
                   op=OP.is_equal)
                tt(out=rqs[:],
                   in0=rlof[:, tsw, None].to_broadcast([128, T_W, 128]),
                   in1=iof[:, None, :].to_broadcast([128, T_W, 128]),
                   op=OP.is_equal)
                tt(out=rwt[:],
                   in0=rhif[:, tsw, None].to_broadcast([128, T_W, RWIN]),
                   in1=iof[:, None, 0:RWIN].to_broadcast([128, T_W, RWIN]),
                   op=OP.is_equal)
                for t in range(T_W):
                    tg = t0 + t
                    csl = slice(t * 128, (t + 1) * 128)
                    ptr = psml.tile([128, 128], bf16, space="PSUM", tag="trn")
                    nc.tensor.transpose(out=ptr[:], in_=ohs[:, t, :],
                                        identity=identb[:])
                    nc.vector.tensor_copy(out=ohg[:, t, :], in_=ptr[:])
                    ptr2 = psml.tile([128, 128], bf16, space="PSUM", tag="trn")
                    nc.tensor.transpose(out=ptr2[:], in_=rqs[:, t, :],
                                        identity=identb[:])
                    nc.vector.tensor_copy(out=rqg[:, t, :], in_=ptr2[:])
                    # bessel + u feature-major
                    pst = psml.tile([32, 128], f32, space="PSUM", tag="sml")
                    nc.tensor.transpose(out=pst[0:8, :], in_=besu[:, tg, :],
                                        identity=ident[:])
                    nc.vector.tensor_copy(out=xfm[32:40, csl],
                                          in_=pst[0:8, :])
                    psu1 = psml.tile([32, 128], f32, space="PSUM", tag="sml")
                    nc.tensor.transpose(out=psu1[0:1, :],
                                        in_=u_pl[:, tg, None],
                                        identity=ident[:])
                    nc.vector.tensor_copy(out=ufm[:, csl], in_=psu1[0:1, :])
                    # endpoint-attr gather: sender (window-local one-hot)
                    gcmb = sp.tile([128, 32], f32, tag="gcmb")
                    psn = psml.tile([128, 32], f32, space="PSUM", tag="sm2")
                    nc.tensor.matmul(
                        out=psn[:, 0:16], lhsT=ohg[:, t, :],
                        rhs=natbf[:, 1024 + w * 16:1024 + (w + 1) * 16],
                        start=True, stop=True)
                    nc.vector.tensor_copy(out=gcmb[:, 0:16], in_=psn[:, 0:16])
                    # receiver: lo-gather matmul then hi-select
                    for c2 in range(2):
                        prg = pgth.tile([128, 512], f32, space="PSUM",
                                        tag="gth")
                        nc.tensor.matmul(
                            out=prg[:], lhsT=rqg[:, t, :],
                            rhs=natv[:, c2 * 512:(c2 + 1) * 512],
                            start=True, stop=True)
                        prod = sp.tile([128, 8, RWIN], f32, tag="rsel")
                        nc.vector.tensor_mul(
                            out=prod[:],
                            in0=prg[:].rearrange("p (a b) -> p a b", b=RWIN),
                            in1=rwt[:, t, None, :].to_broadcast(
                                [128, 8, RWIN]))
                        nc.vector.reduce_sum(
                            out=gcmb[:, 16 + c2 * 8:16 + (c2 + 1) * 8, None],
                            in_=prod[:], axis=AX)
                    ptg = psml.tile([32, 128], f32, space="PSUM", tag="sml")
                    nc.tensor.transpose(out=ptg[:], in_=gcmb[:],
                                        identity=ident[:])
                    nc.vector.tensor_copy(out=xfm[0:32, csl], in_=ptg[:])

                # broadcast u row -> [128, kwin] bf16
                ubc = bgp.tile([128, kwin], bf16)
                for ch in range(NCH):
                    c0 = ch * 512
                    c1 = min(kwin, c0 + 512)
                    psu = pmlp.tile([128, 512], f32, space="PSUM", tag="mlp")
                    nc.tensor.matmul(out=psu[:, :c1 - c0], lhsT=ones_bf[:],
                                     rhs=ufm[:, c0:c1],
                                     start=True, stop=True)
                    nc.vector.tensor_copy(out=ubc[:, c0:c1],
                                          in_=psu[:, :c1 - c0])

                # ---- edge MLP: x0 = u*silu(e1(silu(e0(bes,attrs)))) ----
                x0 = bgp.tile([128, 2, kwin], bf16)
                th = bgp.tile([128, 2, kwin], bf16)
                for ch in range(NCH):
                    c0 = ch * 512
                    c1 = min(kwin, c0 + 512)
                    cw = c1 - c0
                    for hc in range(2):
                        ps = pmlp.tile([128, 512], f32, space="PSUM", tag="mlp")
                        nc.tensor.matmul(
                            out=ps[:, :cw],
                            lhsT=wb[0:40, OFF_WE0 + hc * 128:
                                    OFF_WE0 + (hc + 1) * 128],
                            rhs=xfm[:, c0:c1], start=True, stop=True)
                        silu_act(th[:, hc, c0:c1], ps[:, :cw], bias(0, hc))
                for ch in range(NCH):
                    c0 = ch * 512
                    c1 = min(kwin, c0 + 512)
                    cw = c1 - c0
                    for hc in range(2):
                        ps = pmlp.tile([128, 512], f32, space="PSUM", tag="mlp")
                        for kc in range(2):
                            nc.tensor.matmul(
                                out=ps[:, :cw],
                                lhsT=wb[:, OFF_WE1 + kc * 256 + hc * 128:
                                        OFF_WE1 + kc * 256 + (hc + 1) * 128],
                                rhs=th[:, kc, c0:c1],
                                start=(kc == 0), stop=(kc == 1))
                        silu_act(x0[:, hc, c0:c1], ps[:, :cw], bias(1, hc))
                for hc in range(2):
                    nc.vector.tensor_mul(out=x0[:, hc, :], in0=x0[:, hc, :],
                                          in1=ubc[:])

                # ---- xv, w0 (edge-major [128,16] per tile) ----
                xv = wnp.tile([128, T_W, MUL], f32)
                w0 = wnp.tile([128, T_W, MUL], bf16)
                for t in range(T_W):
                    tsl = slice(t * 128, (t + 1) * 128)
                    p12 = psml.tile([128, 32], f32, space="PSUM", tag="sm2")
                    for kc in range(2):
                        nc.tensor.matmul(
                            out=p12[:, 0:16], lhsT=x0[:, kc, tsl],
                            rhs=wb[:, OFF_WV0 + kc * 16:OFF_WV0 + (kc + 1) * 16],
                            start=(kc == 0), stop=(kc == 1))
                    for kc in range(2):
                        nc.tensor.matmul(
                            out=p12[:, 16:32], lhsT=x0[:, kc, tsl],
                            rhs=wb[:, OFF_WLW0 + kc * 16:
                                    OFF_WLW0 + (kc + 1) * 16],
                            start=(kc == 0), stop=(kc == 1))
                    nc.vector.tensor_copy(out=xv[:, t, :], in_=p12[:, 0:16])
                    nc.vector.tensor_copy(out=w0[:, t, :], in_=p12[:, 16:32])

                # ---- layer-0 scatter: wY[n, m*16+i] ----
                ps_acc = pacc.tile([128, 256], f32, space="PSUM", tag="acc")
                for t in range(T_W):
                    v2 = sp.tile([128, MUL, 16], bf16, tag="v2")
                    nc.vector.tensor_mul(
                        out=v2[:],
                        in0=w0[:, t, :, None].to_broadcast([128, MUL, 16]),
                        in1=Yt[:, t0 + t, None, :].to_broadcast(
                            [128, MUL, 16]))
                    nc.tensor.matmul(
                        out=ps_acc[:],
                        lhsT=ohs[:, t, :],
                        rhs=v2[:].rearrange("p a b -> p (a b)"),
                        start=(t == 0), stop=(t == T_W - 1))
                wY = wnp.tile([128, 256], bf16)
                nc.vector.tensor_copy(out=wY[:], in_=ps_acc[:])

                # ---- gather + Ytil contraction + feedback ----
                # fbfm lives at partitions 64..79 so its matmul shares the
                # base partition of the packed wly1fb_0 weights
                V10 = wnp.tile([128, T_W, MUL], f32)
                fbfm = wnp.tile([80, kwin], bf16)
                prod = wnp.tile([128, MUL, 16], f32)
                ytil = wnp.tile([128, MUL], f32)
                Ssb = wnp.tile([128, MUL], f32)
                fb = wnp.tile([128, MUL], f32)
                for t in range(T_W):
                    pgf = pgth.tile([128, 512], f32, space="PSUM", tag="gth")
                    pg = pgf[:, 0:256]
                    nc.tensor.matmul(out=pg, lhsT=ohg[:, t, :], rhs=wY[:],
                                     start=True, stop=True)
                    pg3 = pg.rearrange("p (a b) -> p a b", b=16)
                    nc.vector.tensor_mul(out=ytil[:], in0=Yt[:, t0 + t, :],
                                          in1=wcol)
                    nc.vector.tensor_mul(
                        out=prod[:], in0=pg3,
                        in1=ytil[:, None, :].to_broadcast([128, MUL, 16]))
                    nc.vector.reduce_sum(out=Ssb[:, :, None], in_=prod[:],
                                         axis=AX)
                    nc.vector.tensor_mul(out=V10[:, t, :], in0=Ssb[:],
                                          in1=xv[:, t, :])
                    nc.vector.tensor_mul(out=fb[:], in0=pg3[:, :, 0],
                                          in1=xv[:, t, :])
                    pst = psml.tile([32, 128], f32, space="PSUM", tag="sml")
                    nc.tensor.transpose(out=pst[0:16, :], in_=fb[:],
                                        identity=ident[:])
                    nc.vector.tensor_copy(
                        out=fbfm[64:80, t * 128:(t + 1) * 128],
                        in_=pst[0:16, :])

                # ---- layer-0 ly1/ly2 + residual -> x1 ----
                x1 = bgp.tile([128, 2, kwin], bf16)

                def mlp_block(xin, xout, l, fbrow, resid_sq2):
                    b1 = OFF_WLY1[l]
                    # wly1fb: layer 0 at rows 64..79 of the WE0 cols,
                    # layer 1 at rows 0..15 of its own FB1 cols
                    fbp, fbc = (64, OFF_WE0) if l == 0 else (0, OFF_FB1)
                    b2 = OFF_WLY2[l]
                    ty = bgp.tile([128, 2, kwin], bf16)
                    for ch in range(NCH):
                        c0 = ch * 512
                        c1 = min(kwin, c0 + 512)
                        cw = c1 - c0
                        for hc in range(2):
                            hs = slice(hc * 128, (hc + 1) * 128)
                            ps = pmlp.tile([128, 512], f32, space="PSUM",
                                           tag="mlp")
                            for kc in range(2):
                                nc.tensor.matmul(
                                    out=ps[:, :cw],
                                    lhsT=wb[:, b1 + kc * 256 + hc * 128:
                                            b1 + kc * 256 + (hc + 1) * 128],
                                    rhs=xin[:, kc, c0:c1],
                                    start=(kc == 0), stop=False)
                            nc.tensor.matmul(
                                out=ps[:, :cw],
                                lhsT=wb[fbp:fbp + 16, fbc + hc * 128:
                                        fbc + (hc + 1) * 128],
                                rhs=fbrow[fbp:fbp + 16, c0:c1],
                                start=False, stop=True)
                            silu_act(ty[:, hc, c0:c1], ps[:, :cw],
                                     bias(2 + l, hc))
                    ty2 = bgp.tile([128, 2, kwin], bf16)
                    for ch in range(NCH):
                        c0 = ch * 512
                        c1 = min(kwin, c0 + 512)
                        cw = c1 - c0
                        for hc in range(2):
                            ps = pmlp.tile([128, 512], f32, space="PSUM",
                                           tag="mlp")
                            for kc in range(2):
                                nc.tensor.matmul(
                                    out=ps[:, :cw],
                                    lhsT=wb[:, b2 + kc * 256 + hc * 128:
                                            b2 + kc * 256 + (hc + 1) * 128],
                                    rhs=ty[:, kc, c0:c1],
                                    start=(kc == 0), stop=(kc == 1))
                            silu_act(ty2[:, hc, c0:c1], ps[:, :cw],
                                     bias(4 + l, hc))
                    # x_out' = x_in' + s * u * y   (s = 1 or sqrt(2))
                    for hc in range(2):
                        nc.vector.tensor_mul(out=ty2[:, hc, :],
                                              in0=ty2[:, hc, :], in1=ubc[:])
                        if resid_sq2:
                            ts(out=ty2[:, hc, :], in0=ty2[:, hc, :],
                               scalar1=math.sqrt(2.0), scalar2=None,
                               op0=OP.mult)
                        nc.vector.tensor_add(out=xout[:, hc, :],
                                             in0=xin[:, hc, :],
                                             in1=ty2[:, hc, :])

                mlp_block(x0, x1, 0, fbfm, False)

                # ---- layer 1: w1, 16-wide scatter/gather, feedback ----
                w1 = wnp.tile([128, T_W, MUL], bf16)
                for t in range(T_W):
                    tsl = slice(t * 128, (t + 1) * 128)
                    p1 = psml.tile([128, 32], f32, space="PSUM", tag="sm2")
                    for kc in range(2):
                        nc.tensor.matmul(
                            out=p1[:, 0:MUL], lhsT=x1[:, kc, tsl],
                            rhs=wb[:, OFF_WLW1 + kc * 16:
                                    OFF_WLW1 + (kc + 1) * 16],
                            start=(kc == 0), stop=(kc == 1))
                    nc.vector.tensor_copy(out=w1[:, t, :], in_=p1[:, 0:MUL])
                ps_a1 = pacc.tile([128, 256], f32, space="PSUM", tag="acc")
                for t in range(T_W):
                    nc.tensor.matmul(out=ps_a1[:, 0:MUL], lhsT=ohs[:, t, :],
                                     rhs=w1[:, t, :],
                                     start=(t == 0), stop=(t == T_W - 1))
                wY1 = wnp.tile([128, MUL], bf16)
                nc.vector.tensor_copy(out=wY1[:], in_=ps_a1[:, 0:MUL])
                fbfm1 = wnp.tile([MUL, kwin], bf16)
                fb1 = wnp.tile([128, MUL], f32)
                for t in range(T_W):
                    pg = pgth.tile([128, 512], f32, space="PSUM", tag="gth")
                    nc.tensor.matmul(out=pg[:, 0:MUL], lhsT=ohg[:, t, :],
                                     rhs=wY1[:], start=True, stop=True)
                    nc.vector.tensor_mul(out=fb1[:], in0=pg[:, 0:MUL],
                                          in1=V10[:, t, :])
                    pst = psml.tile([32, 128], f32, space="PSUM", tag="sml")
                    nc.tensor.transpose(out=pst[0:16, :], in_=fb1[:],
                                        identity=ident[:])
                    nc.vector.tensor_copy(
                        out=fbfm1[:, t * 128:(t + 1) * 128],
                        in_=pst[0:16, :])

                # ---- layer-1 ly1/ly2 + residual -> x2 ----
                x2 = bgp.tile([128, 2, kwin], bf16)
                mlp_block(x1, x2, 1, fbfm1, True)

                # ---- edge out + receiver scatter ----
                eo = wnp.tile([128, 1], f32)
                mt = wnp.tile([128, RWIN], bf16)
                for t in range(T_W):
                    tsl = slice(t * 128, (t + 1) * 128)
                    p1 = psml.tile([128, 32], f32, space="PSUM", tag="sm2")
                    for kc in range(2):
                        nc.tensor.matmul(
                            out=p1[:, 0:1], lhsT=x2[:, kc, tsl],
                            rhs=wb[:, OFF_WOUT + kc:OFF_WOUT + kc + 1],
                            start=(kc == 0), stop=(kc == 1))
                    nc.vector.tensor_mul(out=eo[:], in0=p1[:, 0:1],
                                          in1=u_pl[:, t0 + t, None])
                    nc.vector.tensor_mul(
                        out=mt[:], in0=rwt[:, t, :],
                        in1=eo[:].to_broadcast([128, RWIN]))
                    nc.tensor.matmul(out=ps_rcv[:], lhsT=rqs[:, t, :],
                                     rhs=mt[:],
                                     start=(w == 0 and t == 0),
                                     stop=(w == NW - 1 and t == T_W - 1))

            out_sb = gp.tile([128, RWIN], f32)
            nc.vector.tensor_copy(out=out_sb[:], in_=ps_rcv[:])
            nc.sync.dma_start(out=d_out[:], in_=out_sb[:])

    ET = mybir.EngineType
    eng_map = {ET.DVE: nc.vector, ET.Activation: nc.scalar,
               ET.Pool: nc.gpsimd, ET.PE: nc.tensor, ET.SP: nc.sync}

    def mk_carrier(eng):
        be = eng_map.get(eng)
        if be is None:
            return None
        w = be.wait_ge(carrier_sem, 0)
        ci = w.ins if hasattr(w, "ins") else w
        for bb in nc.m.functions[0].blocks:
            il = list(bb.instructions)
            if any(x is ci for x in il):
                bb.instructions = [x for x in il if x is not ci]
                break
        return ci

    made = _split_waits(nc, mybir, mk_carrier)
    print(f"split_waits: carriers={made}", flush=True)
    return nc


def kernel(**inputs):
    inputs = {k: np.asarray(v) for k, v in inputs.items()}
    kwin, nat_scale, in_maps, _ = make_in_maps(inputs)
    nc = build_graph(kwin, nat_scale)
    from concourse.bass_utils import run_bass_kernel_spmd
    res = run_bass_kernel_spmd(nc, in_maps, core_ids=list(range(NC)))
    out = np.zeros((128, RWIN), np.float64)
    for r in res.results:
        out += np.asarray(r["out"], np.float64)
    # node n = hi*128 + lo stored at [lo, hi]
    return np.ascontiguousarray(out.T.reshape(N, 1)).astype(np.float32)
